# revision 14
# baseline (speedup 1.0000x reference)
"""MLA attention kernel (DeepSeek-style) for 8 Trainium2 NeuronCores.

Sharding: core = b*4 + g*2 + t over (batch b in {0,1}) x (head-group g in
{0,1}: 8 heads each) x (query-fold t in {0,1}).  Each core computes the
full latent pipeline for its batch, q/k/v + attention for its 8 heads and
its 1024 query tokens (two causally-folded 512-blocks), and a partial
output projection; the host sums the two head-group partials.

All tensors flow transposed ([feature-part, token-free]) so no on-chip
transposes are needed: the host supplies x^T per batch, and causal
structure is made SPMD-uniform by permuting the key order per core and
feeding full-block invalidation as per-partition bias columns consumed by
the Exp activation.  All matmuls run as float32r (full-rate fp32).
"""

from contextlib import ExitStack

import numpy as np

import concourse.bacc as bacc
import concourse.bass as bass
import concourse.tile as tile
from concourse import mybir
from concourse.bass_utils import run_bass_kernel_spmd

# Problem shapes (hardcoded per contest contract)
B, S, D = 2, 2048, 2048
H = 16
QL = 1536  # q lora rank
KVL = 512  # kv lora rank
NOPE = 128
ROPE = 64
VD = 128
QKD = NOPE + ROPE  # 192
EPS = 1e-6
SCALE = QKD ** (-0.5)

HPC = 8         # heads per core
NQ = 1024       # query tokens per core
P = 128
NEG = -30000.0  # additive mask value (exp -> 0 in fp32)

F32 = mybir.dt.float32
F32R = mybir.dt.float32r
EXP = mybir.ActivationFunctionType.Exp

N_CORES = 8
SC_A = 8   # key 128-chunks for query block a
SC_B = 16  # key 128-chunks for query block b

ND = D // P        # 16
NRQ = QL // P      # 12
NRKV = KVL // P    # 4
HW = ROPE // 2     # 32


_CACHE = {}


def _rope(nc, pool, out_ap, ps, cos_ap, sin_ap, n):
    """rows 0:32 = even pair elems, 32:64 = odd.
    out[0:32] = e*cos - o*sin ; out[32:64] = e*sin + o*cos."""
    e = ps[0:HW, :]
    o = ps[HW:ROPE, :]
    t1 = pool.tile([HW, n], F32, tag="rp1", name="t1")
    nc.vector.tensor_mul(t1[:], e, cos_ap)
    t2 = pool.tile([HW, n], F32, tag="rp2", name="t2")
    nc.vector.tensor_mul(t2[:], o, sin_ap)
    nc.vector.tensor_sub(out_ap[0:HW, :], t1[:], t2[:])
    t3 = pool.tile([HW, n], F32, tag="rp3", name="t3")
    nc.vector.tensor_mul(t3[:], e, sin_ap)
    t4 = pool.tile([HW, n], F32, tag="rp4", name="t4")
    nc.vector.tensor_mul(t4[:], o, cos_ap)
    nc.vector.tensor_add(out_ap[HW:ROPE, :], t3[:], t4[:])


def build_nc():
    nc = bacc.Bacc("TRN2", target_bir_lowering=False, debug=False,
                   num_devices=N_CORES)

    def inp(name, shape):
        return nc.dram_tensor(name, shape, F32, kind="ExternalInput").ap()

    def inpr(name, shape):
        return nc.dram_tensor(name, shape, F32R, kind="ExternalInput").ap()

    xT = inpr("xT", [D, S])
    wqa = inpr("wq_a", [D, QL])
    wqb = inpr("wq_b", [QL, HPC * QKD])
    wkva = inpr("wkv_a", [D, KVL + ROPE])
    wkvbk = inpr("wkv_b_k", [KVL, HPC * NOPE])
    wkvbv = inpr("wkv_b_v", [KVL, HPC * VD])
    wo = inpr("wo", [HPC * VD, D])
    cosq = inp("cosq", [HW, NQ])
    sinq = inp("sinq", [HW, NQ])
    cosk = inp("cosk", [HW, S])
    sink = inp("sink", [HW, S])
    bias_a = inp("bias_a", [P, SC_A])
    bias_b = inp("bias_b", [P, SC_B])
    out = nc.dram_tensor("out", [NQ, D], F32, kind="ExternalOutput").ap()

    with tile.TileContext(nc) as tc, ExitStack() as ctx, \
            nc.allow_low_precision(reason="fp32r matmul pipeline"):
        const = ctx.enter_context(tc.tile_pool(name="const", bufs=1))
        ones_cf = const.tile([P, 1], F32, tag="ones_cf")
        nc.vector.memset(ones_cf[:], 1.0)
        ones_c = const.tile([P, 1], F32R, tag="ones_c")
        nc.vector.tensor_copy(ones_c[:], ones_cf[:])
        ones_rf = const.tile([1, P], F32, tag="ones_rf")
        nc.vector.memset(ones_rf[:], 1.0)
        ones_r = const.tile([1, P], F32R, tag="ones_r")
        nc.vector.tensor_copy(ones_r[:], ones_rf[:])
        # multiplicative staircase masks (1 keep / 0 drop), applied post-exp
        stairs = []
        for j in range(4):
            st = const.tile([P, 512], F32, tag=f"stair{j}", name=f"st{j}")
            nc.vector.memset(st[:], 1.0)
            # keep 1 where f - p - 128j >= 0 else 0
            nc.gpsimd.affine_select(
                out=st[:], in_=st[:], compare_op=mybir.AluOpType.is_ge,
                fill=0.0, base=-128 * j, pattern=[[1, 512]],
                channel_multiplier=-1)
            stairs.append(st)
        bias_a_sb = const.tile([P, SC_A], F32, tag="bias_a")
        nc.sync.dma_start(bias_a_sb[:], bias_a[:])
        bias_b_sb = const.tile([P, SC_B], F32, tag="bias_b")
        nc.sync.dma_start(bias_b_sb[:], bias_b[:])
        eps_t = const.tile([1, 1], F32, tag="eps")
        nc.vector.memset(eps_t[:], EPS)

        # persistent: kv latents + k_pe (rows 64:128 duplicate rows 0:64 so
        # the scores matmul base-partition matches odd heads' packed qTp)
        latA = ctx.enter_context(tc.tile_pool(name="latA", bufs=1))
        kvT = [latA.tile([P, S], F32R, tag=f"kvT{i}", name=f"kvT{i}")
               for i in range(NRKV)]
        kpe = latA.tile([2 * ROPE, S], F32R, tag="kpe")

        # packed cq latent: 24 [128,512] slices (rc, tbq) in 8 tiles;
        # reused as oTn (and the first wo half) after phase 3
        latQ = ctx.enter_context(tc.tile_pool(name="latQ", bufs=1))
        cqPk = [latQ.tile([P, 1536], F32R, tag=f"cqPk{i}", name=f"cqPk{i}")
                for i in range(8)]

        def cq_slice(rc, tbq, half=None):
            idx = rc * 2 + tbq
            t, c = idx // 3, (idx % 3) * 512
            if half is None:
                return cqPk[t][:, c:c + 512]
            return cqPk[t][:, c + half * 256:c + half * 256 + 256]

        oTn = [cqPk[h][:, 0:NQ] for h in range(HPC)]

        ps_main = ctx.enter_context(
            tc.tile_pool(name="ps_main", bufs=4, space="PSUM"))
        ps_small = ctx.enter_context(
            tc.tile_pool(name="ps_small", bufs=2, space="PSUM"))

        def mm_chain(ps_ap, pairs):
            n = len(pairs)
            for i, (lh, rh) in enumerate(pairs):
                nc.tensor.matmul(ps_ap, lh, rh,
                                 start=(i == 0), stop=(i == n - 1))

        # ---------- Phase 1a: KV A-proj over 8 quarter-strips ----------
        with ExitStack() as p1:
            tabk = p1.enter_context(tc.tile_pool(name="tabk", bufs=2))
            ropep = p1.enter_context(tc.tile_pool(name="ropep", bufs=1))
            wkva_p = p1.enter_context(tc.tile_pool(name="wkva", bufs=1))
            wkva_sb = []
            for dc in range(ND):
                wt = wkva_p.tile([P, KVL + ROPE], F32R, tag=f"wkva{dc}",
                                 name=f"wkva{dc}")
                nc.sync.dma_start(wt[:], wkva[dc * P:(dc + 1) * P, :])
                wkva_sb.append(wt)
            xt_p = p1.enter_context(tc.tile_pool(name="xt", bufs=2))
            for hs in range(8):
                sl = slice(hs * 256, (hs + 1) * 256)
                xts = []
                for dc in range(ND):
                    xt = xt_p.tile([P, 256], F32R, tag=f"xt{dc}",
                                   name=f"xt{dc}")
                    nc.sync.dma_start(xt[:], xT[dc * P:(dc + 1) * P, sl])
                    xts.append(xt)
                for oc in range(NRKV):
                    ps = ps_main.tile([P, 256], F32, tag="ps", name="ps1")
                    mm_chain(ps[:], [
                        (wkva_sb[dc][:, oc * P:(oc + 1) * P], xts[dc][:])
                        for dc in range(ND)])
                    nc.vector.tensor_copy(kvT[oc][:, sl], ps[:])
                psp = ps_main.tile([ROPE, 256], F32, tag="ps", name="ps1p")
                mm_chain(psp[:], [
                    (wkva_sb[dc][:, KVL:KVL + ROPE], xts[dc][:])
                    for dc in range(ND)])
                ck = tabk.tile([HW, 256], F32, tag="cosk", name="ck")
                nc.sync.dma_start(ck[:], cosk[:, sl])
                sk = tabk.tile([HW, 256], F32, tag="sink", name="sk")
                nc.sync.dma_start(sk[:], sink[:, sl])
                _rope(nc, ropep, kpe[0:ROPE, sl], psp, ck[:], sk[:], 256)
                nc.sync.dma_start(kpe[ROPE:2 * ROPE, sl], kpe[0:ROPE, sl])

        # ---------- Phase 1b: Q A-proj (2 wqa col-halves x 4 strips) ------
        with ExitStack() as p1b:
            wqa_p = p1b.enter_context(tc.tile_pool(name="wqa", bufs=1))
            xtq_p = p1b.enter_context(tc.tile_pool(name="xtq", bufs=2))
            for halfW in range(2):
                wqa_sb = []
                for dc in range(ND):
                    wt = wqa_p.tile([P, QL // 2], F32R, tag=f"wqa{dc}",
                                    name=f"wqa{dc}")
                    nc.sync.dma_start(
                        wt[:], wqa[dc * P:(dc + 1) * P,
                                   halfW * (QL // 2):(halfW + 1) * (QL // 2)])
                    wqa_sb.append(wt)
                for hs in range(4):
                    # query cols are local [512:1024] and [1536:2048]
                    lo = (512 if hs < 2 else 1536) + (hs % 2) * 256
                    xts = []
                    for dc in range(ND):
                        xt = xtq_p.tile([P, 256], F32R, tag=f"xtq{dc}",
                                        name=f"xtq{dc}")
                        nc.sync.dma_start(
                            xt[:], xT[dc * P:(dc + 1) * P, lo:lo + 256])
                        xts.append(xt)
                    for oc in range(NRQ // 2):
                        ps = ps_main.tile([P, 256], F32, tag="ps", name="ps1b")
                        mm_chain(ps[:], [
                            (wqa_sb[dc][:, oc * P:(oc + 1) * P], xts[dc][:])
                            for dc in range(ND)])
                        nc.vector.tensor_copy(
                            cq_slice(halfW * (NRQ // 2) + oc,
                                     hs // 2, hs % 2), ps[:])

        # ---------- Phase 2: RMS norms (transposed layout) ----------
        with ExitStack() as p2:
            sqp = p2.enter_context(tc.tile_pool(name="sq", bufs=3))
            rp = p2.enter_context(tc.tile_pool(name="rstd", bufs=1))
            rkv = rp.tile([1, S], F32R, tag="rkv")
            rq = rp.tile([1, NQ], F32R, tag="rq")
            for which, nrc, ntb, rtile, nfeat in (
                    (0, NRKV, 4, rkv, KVL), (1, NRQ, 2, rq, QL)):
                def sl_of(oc, tb):
                    if which == 0:
                        return kvT[oc][:, tb * 512:(tb + 1) * 512]
                    return cq_slice(oc, tb)
                for tb in range(ntb):
                    pss = ps_small.tile([1, 512], F32, tag="pss", name="pss")
                    for oc in range(nrc):
                        sq = sqp.tile([P, 512], F32R, tag="sq", name="sq")
                        nc.vector.tensor_mul(sq[:], sl_of(oc, tb).bitcast(F32),
                                             sl_of(oc, tb).bitcast(F32))
                        nc.tensor.matmul(pss[:], ones_c[:], sq[:],
                                         start=(oc == 0), stop=(oc == nrc - 1))
                    sd = sqp.tile([1, 512], F32, tag="sd", name="sd")
                    nc.scalar.activation(
                        sd[:], pss[:], mybir.ActivationFunctionType.Sqrt,
                        bias=eps_t[:], scale=1.0 / nfeat)
                    nc.vector.reciprocal(
                        rtile[:, tb * 512:(tb + 1) * 512], sd[:])
                for tb in range(ntb):
                    psb = ps_main.tile([P, 512], F32, tag="ps", name="psb")
                    nc.tensor.matmul(
                        psb[:], ones_r[:],
                        rtile[:, tb * 512:(tb + 1) * 512],
                        start=True, stop=True)
                    for oc in range(nrc):
                        nc.vector.tensor_mul(sl_of(oc, tb),
                                             sl_of(oc, tb).bitcast(F32),
                                             psb[:])

        # ---------- Phase 3: qT for all heads ----------
        latQT = ctx.enter_context(tc.tile_pool(name="latQT", bufs=1))
        qTn = [latQT.tile([P, NQ], F32R, tag=f"qTn{h}", name=f"qTn{h}")
               for h in range(HPC)]
        qTpk = [latQT.tile([P, NQ], F32R, tag=f"qTpk{i}", name=f"qTpk{i}")
                for i in range(HPC // 2)]
        qTp = [qTpk[h // 2][(h % 2) * ROPE:(h % 2) * ROPE + ROPE, :]
               for h in range(HPC)]
        with ExitStack() as p3:
            tabq = p3.enter_context(tc.tile_pool(name="tabq", bufs=1))
            cq_sb = tabq.tile([HW, NQ], F32, tag="cosq")
            nc.sync.dma_start(cq_sb[:], cosq[:])
            sq_sb = tabq.tile([HW, NQ], F32, tag="sinq")
            nc.sync.dma_start(sq_sb[:], sinq[:])
            ropep3 = p3.enter_context(tc.tile_pool(name="ropep3", bufs=2))
            wqb_p = p3.enter_context(tc.tile_pool(name="wqb", bufs=2))
            for h in range(HPC):
                wqb_sb = []
                for rc in range(NRQ):
                    wt = wqb_p.tile([P, QKD], F32R, tag=f"wqb{rc}",
                                    name=f"wqb{rc}")
                    nc.sync.dma_start(
                        wt[:], wqb[rc * P:(rc + 1) * P, h * QKD:(h + 1) * QKD])
                    wqb_sb.append(wt)
                for tbq in range(2):
                    sl = slice(tbq * 512, (tbq + 1) * 512)
                    ps = ps_main.tile([P, 512], F32, tag="ps", name="ps3")
                    mm_chain(ps[:], [
                        (wqb_sb[rc][:, :NOPE], cq_slice(rc, tbq))
                        for rc in range(NRQ)])
                    nc.vector.tensor_copy(qTn[h][:, sl], ps[:])
                    psp = ps_main.tile([ROPE, 512], F32, tag="ps", name="ps3p")
                    mm_chain(psp[:], [
                        (wqb_sb[rc][:, NOPE:QKD], cq_slice(rc, tbq))
                        for rc in range(NRQ)])
                    _rope(nc, ropep3, qTp[h][:, sl], psp,
                          cq_sb[:, sl], sq_sb[:, sl], 512)

        # ---------- Phase 4: attention per head-pair ----------
        with ExitStack() as p4:
            kt_p = p4.enter_context(tc.tile_pool(name="kt", bufs=2))
            v_p = p4.enter_context(tc.tile_pool(name="v", bufs=1))
            wk_p = p4.enter_context(tc.tile_pool(name="wkvb", bufs=1))
            work = p4.enter_context(tc.tile_pool(name="work", bufs=3))
            acc_p = p4.enter_context(tc.tile_pool(name="acc", bufs=2))
            ps_o = p4.enter_context(
                tc.tile_pool(name="ps_o", bufs=2, space="PSUM"))
            for hp in range(HPC // 2):
                heads = (2 * hp, 2 * hp + 1)
                kT = {}
                for h in heads:
                    wk_sb = []
                    for rc in range(NRKV):
                        wt = wk_p.tile([P, NOPE], F32R, tag=f"wkvbk{rc}",
                                       name=f"wkk{rc}")
                        nc.sync.dma_start(
                            wt[:], wkvbk[rc * P:(rc + 1) * P,
                                         h * NOPE:(h + 1) * NOPE])
                        wk_sb.append(wt)
                    kt = kt_p.tile([P, S], F32R, tag="kt", name=f"kt{h}")
                    for tb in range(4):
                        sl = slice(tb * 512, (tb + 1) * 512)
                        ps = ps_main.tile([P, 512], F32, tag="ps", name="ps4k")
                        mm_chain(ps[:], [(wk_sb[rc][:], kvT[rc][:, sl])
                                         for rc in range(NRKV)])
                        nc.vector.tensor_copy(kt[:, sl], ps[:])
                    kT[h] = kt
                wv_sb = []
                for rc in range(NRKV):
                    wt = wk_p.tile([P, 2 * VD], F32R, tag=f"wkvbv{rc}",
                                   name=f"wkv{rc}")
                    nc.sync.dma_start(
                        wt[:], wkvbv[rc * P:(rc + 1) * P,
                                     heads[0] * VD:(heads[0] + 2) * VD])
                    wv_sb.append(wt)
                vt = v_p.tile([P, 16 * 2 * VD], F32R, tag="vt", name="vt")
                for tk in range(16):
                    ps = ps_main.tile([P, 2 * VD], F32, tag="ps", name="ps4v")
                    mm_chain(ps[:], [
                        (kvT[rc][:, tk * P:(tk + 1) * P], wv_sb[rc][:])
                        for rc in range(NRKV)])
                    nc.vector.tensor_copy(
                        vt[:, tk * 2 * VD:(tk + 1) * 2 * VD], ps[:])

                for h in heads:
                    hv = h % 2
                    for qb, (nsc, bias_sb) in enumerate(
                            ((SC_A, bias_a_sb), (SC_B, bias_b_sb))):
                        q0 = qb * 512
                        oT = ps_o.tile([P, 512], F32, tag="oT", name="oT")
                        acc = acc_p.tile([P, 512], F32R, tag="acc", name="acc")
                        for sc in range(nsc):
                            sps = ps_main.tile([P, 512], F32, tag="ps",
                                               name="ps4s")
                            nc.tensor.matmul(
                                sps[:], kT[h][:, sc * P:(sc + 1) * P],
                                qTn[h][:, q0:q0 + 512],
                                start=True, stop=False)
                            nc.tensor.matmul(
                                sps[:],
                                kpe[hv * ROPE:(hv + 1) * ROPE,
                                    sc * P:(sc + 1) * P],
                                qTp[h][:, q0:q0 + 512],
                                start=False, stop=True)
                            pt = work.tile([P, 512], F32R, tag="pt", name="pt")
                            jd = sc - (nsc - 4)
                            if jd >= 0:
                                nc.scalar.activation(pt[:], sps[:], EXP)
                                nc.vector.tensor_mul(pt[:],
                                                     pt[:].bitcast(F32),
                                                     stairs[jd][:])
                            else:
                                nc.scalar.activation(
                                    pt[:], sps[:], EXP,
                                    bias=bias_sb[:, sc:sc + 1])
                            if sc == 0:
                                nc.vector.tensor_copy(acc[:],
                                                      pt[:].bitcast(F32))
                            else:
                                nc.vector.tensor_add(acc[:],
                                                     acc[:].bitcast(F32),
                                                     pt[:].bitcast(F32))
                            nc.tensor.matmul(
                                oT[:],
                                vt[:, sc * 2 * VD + hv * VD:
                                      sc * 2 * VD + (hv + 1) * VD],
                                pt[:], start=(sc == 0),
                                stop=(sc == nsc - 1))
                        pss = ps_small.tile([1, 512], F32, tag="pss",
                                            name="pss4")
                        nc.tensor.matmul(pss[:], ones_c[:], acc[:],
                                         start=True, stop=True)
                        rs = work.tile([1, 512], F32R, tag="rs", name="rs")
                        nc.vector.reciprocal(rs[:], pss[:])
                        psb = ps_main.tile([P, 512], F32, tag="ps",
                                           name="ps4b")
                        nc.tensor.matmul(psb[:], ones_r[:], rs[:],
                                         start=True, stop=True)
                        bc = work.tile([P, 512], F32, tag="bc", name="bc")
                        nc.scalar.copy(bc[:], psb[:])
                        nc.vector.tensor_mul(oTn[h][:, q0:q0 + 512],
                                             oT[:], bc[:])

        # ---------- Phase 5: output projection (wo aliases qTn/kvT) -------
        with ExitStack() as p5:
            os_p = p5.enter_context(tc.tile_pool(name="os", bufs=4))
            wo_half = []  # per head: (half0 ap, half1 ap), each [128, 1024]
            for h in range(HPC):
                h0 = qTn[h][:, :]
                h1 = kvT[h // 2][:, (h % 2) * 1024:(h % 2) * 1024 + 1024]
                nc.sync.dma_start(h0, wo[h * P:(h + 1) * P, 0:1024])
                nc.sync.dma_start(h1, wo[h * P:(h + 1) * P, 1024:2048])
                wo_half.append((h0, h1))
            for tk in range(NQ // P):
                for dcb in range(4):
                    ps = ps_main.tile([P, 512], F32, tag="ps", name="ps5")
                    for h in range(HPC):
                        src = wo_half[h][dcb // 2]
                        rh = src[:, (dcb % 2) * 512:(dcb % 2) * 512 + 512]
                        nc.tensor.matmul(
                            ps[:], oTn[h][:, tk * P:(tk + 1) * P],
                            rh,
                            start=(h == 0), stop=(h == HPC - 1))
                    ot = os_p.tile([P, 512], F32, tag="ot", name="ot")
                    nc.scalar.copy(ot[:], ps[:])
                    nc.sync.dma_start(
                        out[tk * P:(tk + 1) * P,
                            dcb * 512:(dcb + 1) * 512], ot[:])

    nc.compile()
    return nc


def _prep_inputs(x, freqs_cis, wq_a, q_norm_w, wq_b, wkv_a, kv_norm_w,
                 wkv_b, wo):
    """Host-side shard prep. Returns (in_maps, meta) for 8 cores."""
    x = np.asarray(x, np.float32)
    freqs_cis = np.asarray(freqs_cis, np.float32)
    wq_a = np.ascontiguousarray(np.asarray(wq_a, np.float32))
    q_norm_w = np.asarray(q_norm_w, np.float32)
    wq_b = np.asarray(wq_b, np.float32)
    wkv_a = np.asarray(wkv_a, np.float32)
    kv_norm_w = np.asarray(kv_norm_w, np.float32)
    wkv_b = np.asarray(wkv_b, np.float32)
    wo = np.asarray(wo, np.float32)

    # de-interleave perm for rope pairs: [e0..e31, o0..o31]
    perm = np.concatenate([np.arange(0, ROPE, 2), np.arange(1, ROPE, 2)])

    wqb = (wq_b * q_norm_w[:, None] * SCALE).reshape(QL, H, QKD)
    wqb = np.concatenate(
        [wqb[:, :, :NOPE], wqb[:, :, NOPE:][:, :, perm]], axis=2)

    wkva = np.ascontiguousarray(np.concatenate(
        [wkv_a[:, :KVL], wkv_a[:, KVL:][:, perm]], axis=1))

    wkvb = (wkv_b * kv_norm_w[:, None]).reshape(KVL, H, NOPE + VD)
    wkvb_k = wkvb[:, :, :NOPE]
    wkvb_v = wkvb[:, :, NOPE:]

    cos_t = np.ascontiguousarray(freqs_cis[:, :, 0].T)  # [32, S]
    sin_t = np.ascontiguousarray(freqs_cis[:, :, 1].T)

    sig0 = np.arange(S)
    sig1 = np.concatenate([sig0[512:1024], sig0[0:512],
                           sig0[1536:2048], sig0[1024:1536]])
    qpos = {0: np.concatenate([sig0[512:1024], sig0[1536:2048]]),
            1: np.concatenate([sig0[0:512], sig0[1024:1536]])}

    bias_a0 = np.zeros((P, SC_A), np.float32)
    bias_b0 = np.zeros((P, SC_B), np.float32)
    bias_a1 = np.zeros((P, SC_A), np.float32)
    bias_a1[:, 0:4] = NEG
    bias_b1 = np.zeros((P, SC_B), np.float32)
    bias_b1[:, 8:12] = NEG

    in_maps = []
    meta = []
    for c in range(N_CORES):
        b, g, t = c // 4, (c // 2) % 2, c % 2
        sig = sig0 if t == 0 else sig1
        hs = slice(g * HPC, (g + 1) * HPC)
        m = {
            "xT": np.ascontiguousarray(x[b].T[:, sig]),
            "wq_a": wq_a,
            "wq_b": np.ascontiguousarray(
                wqb[:, hs, :].reshape(QL, HPC * QKD)),
            "wkv_a": wkva,
            "wkv_b_k": np.ascontiguousarray(
                wkvb_k[:, hs, :].reshape(KVL, HPC * NOPE)),
            "wkv_b_v": np.ascontiguousarray(
                wkvb_v[:, hs, :].reshape(KVL, HPC * VD)),
            "wo": np.ascontiguousarray(wo[g * HPC * VD:(g + 1) * HPC * VD, :]),
            "cosq": np.ascontiguousarray(cos_t[:, qpos[t]]),
            "sinq": np.ascontiguousarray(sin_t[:, qpos[t]]),
            "cosk": np.ascontiguousarray(cos_t[:, sig]),
            "sink": np.ascontiguousarray(sin_t[:, sig]),
            "bias_a": bias_a0 if t == 0 else bias_a1,
            "bias_b": bias_b0 if t == 0 else bias_b1,
        }
        in_maps.append(m)
        meta.append((b, g, t))
    return in_maps, meta


def kernel(**inputs):
    in_maps, meta = _prep_inputs(**inputs)
    if "nc" not in _CACHE:
        _CACHE["nc"] = build_nc()
    nc = _CACHE["nc"]
    res = run_bass_kernel_spmd(nc, in_maps, core_ids=list(range(N_CORES)),
                               **_CACHE.get("run_kwargs", {}))
    _CACHE["last_result"] = res
    out = np.zeros((B, S, D), np.float32)
    for c in range(N_CORES):
        b, g, t = meta[c]
        part = res.results[c]["out"]  # [1024, 2048]
        if t == 0:
            out[b, 512:1024] += part[:512]
            out[b, 1536:2048] += part[512:]
        else:
            out[b, 0:512] += part[:512]
            out[b, 1024:1536] += part[512:]
    return out


# revision 15
# speedup vs baseline: 1.3208x; 1.3208x over previous
"""MLA attention kernel (DeepSeek-style) for 8 Trainium2 NeuronCores.

Sharding: core = b*4 + g*2 + t over (batch b in {0,1}) x (head-group g in
{0,1}: 8 heads each) x (query-fold t in {0,1}).  Each core computes the
full latent pipeline for its batch, q/k/v + attention for its 8 heads and
its 1024 query tokens (two causally-folded 512-blocks), and a partial
output projection; the host sums the two head-group partials.

All tensors flow transposed ([feature-part, token-free]) so no on-chip
transposes are needed: the host supplies x^T per batch, and causal
structure is made SPMD-uniform by permuting the key order per core and
feeding full-block invalidation as per-partition bias columns consumed by
the Exp activation.  Matmul operands are fp16 (1 cyc/row on the PE; all
values are O(1) so fp16's 11-bit mantissa gives ~5e-4 rounding).
"""

from contextlib import ExitStack

import numpy as np

import concourse.bacc as bacc
import concourse.bass as bass
import concourse.tile as tile
from concourse import mybir
from concourse.bass_utils import run_bass_kernel_spmd

# Problem shapes (hardcoded per contest contract)
B, S, D = 2, 2048, 2048
H = 16
QL = 1536  # q lora rank
KVL = 512  # kv lora rank
NOPE = 128
ROPE = 64
VD = 128
QKD = NOPE + ROPE  # 192
EPS = 1e-6
SCALE = QKD ** (-0.5)

HPC = 8         # heads per core
NQ = 1024       # query tokens per core
P = 128
NEG = -30000.0  # additive mask value (exp -> 0)

F32 = mybir.dt.float32
F16 = mybir.dt.float16
EXP = mybir.ActivationFunctionType.Exp

N_CORES = 8
SC_A = 8   # key 128-chunks for query block a
SC_B = 16  # key 128-chunks for query block b

ND = D // P        # 16
NRQ = QL // P      # 12
NRKV = KVL // P    # 4
HW = ROPE // 2     # 32

_CACHE = {}


def _rope(nc, pool, out_ap, ps, cos_ap, sin_ap, n):
    """rows 0:32 = even pair elems, 32:64 = odd.
    out[0:32] = e*cos - o*sin ; out[32:64] = e*sin + o*cos."""
    e = ps[0:HW, :]
    o = ps[HW:ROPE, :]
    t1 = pool.tile([HW, n], F32, tag="rp1", name="t1")
    nc.vector.tensor_mul(t1[:], e, cos_ap)
    t2 = pool.tile([HW, n], F32, tag="rp2", name="t2")
    nc.vector.tensor_mul(t2[:], o, sin_ap)
    nc.vector.tensor_sub(out_ap[0:HW, :], t1[:], t2[:])
    t3 = pool.tile([HW, n], F32, tag="rp3", name="t3")
    nc.vector.tensor_mul(t3[:], e, sin_ap)
    t4 = pool.tile([HW, n], F32, tag="rp4", name="t4")
    nc.vector.tensor_mul(t4[:], o, cos_ap)
    nc.vector.tensor_add(out_ap[HW:ROPE, :], t3[:], t4[:])


def build_nc():
    nc = bacc.Bacc("TRN2", target_bir_lowering=False, debug=False,
                   num_devices=N_CORES)

    def inp(name, shape, dt=F32):
        return nc.dram_tensor(name, shape, dt, kind="ExternalInput").ap()

    xT = inp("xT", [D, S], F16)
    wqa = inp("wq_a", [D, QL], F16)
    wqb = inp("wq_b", [QL, HPC * QKD], F16)
    wkva = inp("wkv_a", [D, KVL + ROPE], F16)
    wkvbk = inp("wkv_b_k", [KVL, HPC * NOPE], F16)
    wkvbv = inp("wkv_b_v", [KVL, HPC * VD], F16)
    wo = inp("wo", [HPC * VD, D], F16)
    cosq = inp("cosq", [HW, NQ])
    sinq = inp("sinq", [HW, NQ])
    cosk = inp("cosk", [HW, S])
    sink = inp("sink", [HW, S])
    bias_a = inp("bias_a", [P, SC_A])
    bias_b = inp("bias_b", [P, SC_B])
    out = nc.dram_tensor("out", [NQ, D], F32, kind="ExternalOutput").ap()

    with tile.TileContext(nc) as tc, ExitStack() as ctx, \
            nc.allow_low_precision(reason="fp16 matmul pipeline"):
        const = ctx.enter_context(tc.tile_pool(name="const", bufs=1))
        ones_cf = const.tile([P, 1], F32, tag="ones_cf")
        nc.vector.memset(ones_cf[:], 1.0)
        ones_c = const.tile([P, 1], F16, tag="ones_c")
        nc.vector.tensor_copy(ones_c[:], ones_cf[:])
        ones_rf = const.tile([1, P], F32, tag="ones_rf")
        nc.vector.memset(ones_rf[:], 1.0)
        ones_r = const.tile([1, P], F16, tag="ones_r")
        nc.vector.tensor_copy(ones_r[:], ones_rf[:])
        # multiplicative staircase masks (1 keep / 0 drop), applied post-exp
        stairs = []
        for j in range(4):
            st = const.tile([P, 512], F32, tag=f"stair{j}", name=f"st{j}")
            nc.vector.memset(st[:], 1.0)
            # keep 1 where f - p - 128j >= 0 else 0
            nc.gpsimd.affine_select(
                out=st[:], in_=st[:], compare_op=mybir.AluOpType.is_ge,
                fill=0.0, base=-128 * j, pattern=[[1, 512]],
                channel_multiplier=-1)
            stairs.append(st)
        bias_a_sb = const.tile([P, SC_A], F32, tag="bias_a")
        nc.sync.dma_start(bias_a_sb[:], bias_a[:])
        bias_b_sb = const.tile([P, SC_B], F32, tag="bias_b")
        nc.sync.dma_start(bias_b_sb[:], bias_b[:])
        eps_t = const.tile([1, 1], F32, tag="eps")
        nc.vector.memset(eps_t[:], EPS)

        # persistent: kv latents + k_pe (rows 64:128 duplicate rows 0:64 so
        # the scores matmul base-partition matches odd heads' packed qTp)
        latA = ctx.enter_context(tc.tile_pool(name="latA", bufs=1))
        kvT = [latA.tile([P, S], F16, tag=f"kvT{i}", name=f"kvT{i}")
               for i in range(NRKV)]
        kpe = latA.tile([2 * ROPE, S], F16, tag="kpe")

        # packed cq latent: 24 [128,512] slices (rc, tbq) in 8 tiles;
        # reused as oTn after phase 3
        latQ = ctx.enter_context(tc.tile_pool(name="latQ", bufs=1))
        cqPk = [latQ.tile([P, 1536], F16, tag=f"cqPk{i}", name=f"cqPk{i}")
                for i in range(8)]

        def cq_slice(rc, tbq):
            idx = rc * 2 + tbq
            t, c = idx // 3, (idx % 3) * 512
            return cqPk[t][:, c:c + 512]

        oTn = [cqPk[h][:, 0:NQ] for h in range(HPC)]

        ps_main = ctx.enter_context(
            tc.tile_pool(name="ps_main", bufs=4, space="PSUM"))
        ps_small = ctx.enter_context(
            tc.tile_pool(name="ps_small", bufs=2, space="PSUM"))

        def mm_chain(ps_ap, pairs):
            n = len(pairs)
            for i, (lh, rh) in enumerate(pairs):
                nc.tensor.matmul(ps_ap, lh, rh,
                                 start=(i == 0), stop=(i == n - 1))

        # ---------- Phase 1a: KV A-proj over 4 key strips ----------
        with ExitStack() as p1:
            tabk = p1.enter_context(tc.tile_pool(name="tabk", bufs=2))
            ropep = p1.enter_context(tc.tile_pool(name="ropep", bufs=2))
            wkva_p = p1.enter_context(tc.tile_pool(name="wkva", bufs=1))
            wkva_sb = []
            for dc in range(ND):
                wt = wkva_p.tile([P, KVL + ROPE], F16, tag=f"wkva{dc}",
                                 name=f"wkva{dc}")
                nc.sync.dma_start(wt[:], wkva[dc * P:(dc + 1) * P, :])
                wkva_sb.append(wt)
            xt_p = p1.enter_context(tc.tile_pool(name="xt", bufs=2))
            for hs in range(4):
                sl = slice(hs * 512, (hs + 1) * 512)
                xts = []
                for dc in range(ND):
                    xt = xt_p.tile([P, 512], F16, tag=f"xt{dc}",
                                   name=f"xt{dc}")
                    nc.sync.dma_start(xt[:], xT[dc * P:(dc + 1) * P, sl])
                    xts.append(xt)
                for oc in range(NRKV):
                    ps = ps_main.tile([P, 512], F32, tag="ps", name="ps1")
                    mm_chain(ps[:], [
                        (wkva_sb[dc][:, oc * P:(oc + 1) * P], xts[dc][:])
                        for dc in range(ND)])
                    nc.vector.tensor_copy(kvT[oc][:, sl], ps[:])
                psp = ps_main.tile([ROPE, 512], F32, tag="ps", name="ps1p")
                mm_chain(psp[:], [
                    (wkva_sb[dc][:, KVL:KVL + ROPE], xts[dc][:])
                    for dc in range(ND)])
                ck = tabk.tile([HW, 512], F32, tag="cosk", name="ck")
                nc.sync.dma_start(ck[:], cosk[:, sl])
                sk = tabk.tile([HW, 512], F32, tag="sink", name="sk")
                nc.sync.dma_start(sk[:], sink[:, sl])
                _rope(nc, ropep, kpe[0:ROPE, sl], psp, ck[:], sk[:], 512)
                nc.sync.dma_start(kpe[ROPE:2 * ROPE, sl], kpe[0:ROPE, sl])

        # ---------- Phase 1b: Q A-proj (full wq_a, 2 query strips) --------
        with ExitStack() as p1b:
            wqa_p = p1b.enter_context(tc.tile_pool(name="wqa", bufs=1))
            xtq_p = p1b.enter_context(tc.tile_pool(name="xtq", bufs=2))
            wqa_sb = []
            for dc in range(ND):
                wt = wqa_p.tile([P, QL], F16, tag=f"wqa{dc}",
                                name=f"wqa{dc}")
                nc.sync.dma_start(wt[:], wqa[dc * P:(dc + 1) * P, :])
                wqa_sb.append(wt)
            for hs in range(2):
                lo = 512 if hs == 0 else 1536
                xts = []
                for dc in range(ND):
                    xt = xtq_p.tile([P, 512], F16, tag=f"xtq{dc}",
                                    name=f"xtq{dc}")
                    nc.sync.dma_start(
                        xt[:], xT[dc * P:(dc + 1) * P, lo:lo + 512])
                    xts.append(xt)
                for oc in range(NRQ):
                    ps = ps_main.tile([P, 512], F32, tag="ps", name="ps1b")
                    mm_chain(ps[:], [
                        (wqa_sb[dc][:, oc * P:(oc + 1) * P], xts[dc][:])
                        for dc in range(ND)])
                    nc.vector.tensor_copy(cq_slice(oc, hs), ps[:])

        # ---------- Phase 2: RMS norms (transposed layout) ----------
        with ExitStack() as p2:
            sqp = p2.enter_context(tc.tile_pool(name="sq", bufs=3))
            for which, nrc, ntb, nfeat in (
                    (0, NRKV, 4, KVL), (1, NRQ, 2, QL)):
                def sl_of(oc, tb):
                    if which == 0:
                        return kvT[oc][:, tb * 512:(tb + 1) * 512]
                    return cq_slice(oc, tb)
                for tb in range(ntb):
                    pss = ps_small.tile([1, 512], F32, tag="pss", name="pss")
                    for oc in range(nrc):
                        sq = sqp.tile([P, 512], F16, tag="sq", name="sq")
                        nc.vector.tensor_mul(sq[:], sl_of(oc, tb),
                                             sl_of(oc, tb))
                        nc.tensor.matmul(pss[:], ones_c[:], sq[:],
                                         start=(oc == 0), stop=(oc == nrc - 1))
                    sd = sqp.tile([1, 512], F16, tag="sd", name="sd")
                    nc.scalar.activation(
                        sd[:], pss[:], mybir.ActivationFunctionType.Sqrt,
                        bias=eps_t[:], scale=1.0 / nfeat)
                    psb = ps_main.tile([P, 512], F32, tag="ps", name="psb")
                    nc.tensor.matmul(psb[:], ones_r[:], sd[:],
                                     start=True, stop=True)
                    rb = sqp.tile([P, 512], F16, tag="rb", name="rb")
                    nc.vector.reciprocal(rb[:], psb[:])
                    for oc in range(nrc):
                        nc.vector.tensor_mul(sl_of(oc, tb), sl_of(oc, tb),
                                             rb[:])

        # ---------- Phase 3: qT for all heads ----------
        latQT = ctx.enter_context(tc.tile_pool(name="latQT", bufs=1))
        qTn = [latQT.tile([P, NQ], F16, tag=f"qTn{h}", name=f"qTn{h}")
               for h in range(HPC)]
        qTpk = [latQT.tile([P, NQ], F16, tag=f"qTpk{i}", name=f"qTpk{i}")
                for i in range(HPC // 2)]
        qTp = [qTpk[h // 2][(h % 2) * ROPE:(h % 2) * ROPE + ROPE, :]
               for h in range(HPC)]
        with ExitStack() as p3:
            tabq = p3.enter_context(tc.tile_pool(name="tabq", bufs=1))
            cq_sb = tabq.tile([HW, NQ], F32, tag="cosq")
            nc.sync.dma_start(cq_sb[:], cosq[:])
            sq_sb = tabq.tile([HW, NQ], F32, tag="sinq")
            nc.sync.dma_start(sq_sb[:], sinq[:])
            ropep3 = p3.enter_context(tc.tile_pool(name="ropep3", bufs=2))
            wqb_p = p3.enter_context(tc.tile_pool(name="wqb", bufs=3))
            for h in range(HPC):
                wqb_sb = []
                for rc in range(NRQ):
                    wt = wqb_p.tile([P, QKD], F16, tag=f"wqb{rc}",
                                    name=f"wqb{rc}")
                    nc.sync.dma_start(
                        wt[:], wqb[rc * P:(rc + 1) * P, h * QKD:(h + 1) * QKD])
                    wqb_sb.append(wt)
                for tbq in range(2):
                    sl = slice(tbq * 512, (tbq + 1) * 512)
                    ps = ps_main.tile([P, 512], F32, tag="ps", name="ps3")
                    mm_chain(ps[:], [
                        (wqb_sb[rc][:, :NOPE], cq_slice(rc, tbq))
                        for rc in range(NRQ)])
                    nc.vector.tensor_copy(qTn[h][:, sl], ps[:])
                    psp = ps_main.tile([ROPE, 512], F32, tag="ps", name="ps3p")
                    mm_chain(psp[:], [
                        (wqb_sb[rc][:, NOPE:QKD], cq_slice(rc, tbq))
                        for rc in range(NRQ)])
                    _rope(nc, ropep3, qTp[h][:, sl], psp,
                          cq_sb[:, sl], sq_sb[:, sl], 512)

        # ---------- Phase 4: attention per head-pair ----------
        with ExitStack() as p4:
            kt_p = p4.enter_context(tc.tile_pool(name="kt", bufs=4))
            v_p = p4.enter_context(tc.tile_pool(name="v", bufs=2))
            wk_p = p4.enter_context(tc.tile_pool(name="wkvb", bufs=2))
            work = p4.enter_context(tc.tile_pool(name="work", bufs=4))
            acc_p = p4.enter_context(tc.tile_pool(name="acc", bufs=2))
            ps_o = p4.enter_context(
                tc.tile_pool(name="ps_o", bufs=2, space="PSUM"))
            for hp in range(HPC // 2):
                heads = (2 * hp, 2 * hp + 1)
                kT = {}
                for h in heads:
                    wk_sb = []
                    for rc in range(NRKV):
                        wt = wk_p.tile([P, NOPE], F16, tag=f"wkvbk{rc}",
                                       name=f"wkk{rc}")
                        nc.sync.dma_start(
                            wt[:], wkvbk[rc * P:(rc + 1) * P,
                                         h * NOPE:(h + 1) * NOPE])
                        wk_sb.append(wt)
                    kt = kt_p.tile([P, S], F16, tag="kt", name=f"kt{h}")
                    for tb in range(4):
                        sl = slice(tb * 512, (tb + 1) * 512)
                        ps = ps_main.tile([P, 512], F32, tag="ps", name="ps4k")
                        mm_chain(ps[:], [(wk_sb[rc][:], kvT[rc][:, sl])
                                         for rc in range(NRKV)])
                        nc.vector.tensor_copy(kt[:, sl], ps[:])
                    kT[h] = kt
                wv_sb = []
                for rc in range(NRKV):
                    wt = wk_p.tile([P, 2 * VD], F16, tag=f"wkvbv{rc}",
                                   name=f"wkv{rc}")
                    nc.sync.dma_start(
                        wt[:], wkvbv[rc * P:(rc + 1) * P,
                                     heads[0] * VD:(heads[0] + 2) * VD])
                    wv_sb.append(wt)
                vt = v_p.tile([P, 16 * 2 * VD], F16, tag="vt", name="vt")
                for tk in range(16):
                    ps = ps_main.tile([P, 2 * VD], F32, tag="ps", name="ps4v")
                    mm_chain(ps[:], [
                        (kvT[rc][:, tk * P:(tk + 1) * P], wv_sb[rc][:])
                        for rc in range(NRKV)])
                    nc.vector.tensor_copy(
                        vt[:, tk * 2 * VD:(tk + 1) * 2 * VD], ps[:])

                for h in heads:
                    hv = h % 2
                    for qb, (nsc, bias_sb) in enumerate(
                            ((SC_A, bias_a_sb), (SC_B, bias_b_sb))):
                        q0 = qb * 512
                        oT = ps_o.tile([P, 512], F32, tag="oT", name="oT")
                        acc = acc_p.tile([P, 512], F16, tag="acc", name="acc")
                        for sc in range(nsc):
                            sps = ps_main.tile([P, 512], F32, tag="ps",
                                               name="ps4s")
                            nc.tensor.matmul(
                                sps[:], kT[h][:, sc * P:(sc + 1) * P],
                                qTn[h][:, q0:q0 + 512],
                                start=True, stop=False)
                            nc.tensor.matmul(
                                sps[:],
                                kpe[hv * ROPE:(hv + 1) * ROPE,
                                    sc * P:(sc + 1) * P],
                                qTp[h][:, q0:q0 + 512],
                                start=False, stop=True)
                            pt = work.tile([P, 512], F16, tag="pt", name="pt")
                            jd = sc - (nsc - 4)
                            if jd >= 0:
                                nc.scalar.activation(pt[:], sps[:], EXP)
                                nc.vector.tensor_mul(pt[:], pt[:],
                                                     stairs[jd][:])
                            else:
                                nc.scalar.activation(
                                    pt[:], sps[:], EXP,
                                    bias=bias_sb[:, sc:sc + 1])
                            if sc == 0:
                                nc.vector.tensor_copy(acc[:], pt[:])
                            else:
                                nc.vector.tensor_add(acc[:], acc[:], pt[:])
                            nc.tensor.matmul(
                                oT[:],
                                vt[:, sc * 2 * VD + hv * VD:
                                   sc * 2 * VD + (hv + 1) * VD],
                                pt[:], start=(sc == 0),
                                stop=(sc == nsc - 1))
                        pss = ps_small.tile([1, 512], F32, tag="pss",
                                            name="pss4")
                        nc.tensor.matmul(pss[:], ones_c[:], acc[:],
                                         start=True, stop=True)
                        ss = work.tile([1, 512], F16, tag="ss", name="ss")
                        nc.scalar.copy(ss[:], pss[:])
                        psb = ps_main.tile([P, 512], F32, tag="ps",
                                           name="ps4b")
                        nc.tensor.matmul(psb[:], ones_r[:], ss[:],
                                         start=True, stop=True)
                        rb = work.tile([P, 512], F16, tag="rb", name="rb")
                        nc.vector.reciprocal(rb[:], psb[:])
                        nc.vector.tensor_mul(oTn[h][:, q0:q0 + 512],
                                             oT[:], rb[:])

        # ---------- Phase 5: output projection (wo aliases qTn/kvT) -------
        with ExitStack() as p5:
            os_p = p5.enter_context(tc.tile_pool(name="os", bufs=4))
            wo_half = []  # per head: (half0 ap, half1 ap), each [128, 1024]
            for h in range(HPC):
                h0 = qTn[h][:, :]
                h1 = kvT[h // 2][:, (h % 2) * 1024:(h % 2) * 1024 + 1024]
                nc.sync.dma_start(h0, wo[h * P:(h + 1) * P, 0:1024])
                nc.sync.dma_start(h1, wo[h * P:(h + 1) * P, 1024:2048])
                wo_half.append((h0, h1))
            for tk in range(NQ // P):
                for dcb in range(4):
                    ps = ps_main.tile([P, 512], F32, tag="ps", name="ps5")
                    for h in range(HPC):
                        src = wo_half[h][dcb // 2]
                        rh = src[:, (dcb % 2) * 512:(dcb % 2) * 512 + 512]
                        nc.tensor.matmul(
                            ps[:], oTn[h][:, tk * P:(tk + 1) * P], rh,
                            start=(h == 0), stop=(h == HPC - 1))
                    ot = os_p.tile([P, 512], F32, tag="ot", name="ot")
                    nc.scalar.copy(ot[:], ps[:])
                    nc.sync.dma_start(
                        out[tk * P:(tk + 1) * P,
                            dcb * 512:(dcb + 1) * 512], ot[:])

    nc.compile()
    return nc


def _prep_inputs(x, freqs_cis, wq_a, q_norm_w, wq_b, wkv_a, kv_norm_w,
                 wkv_b, wo):
    """Host-side shard prep. Returns (in_maps, meta) for 8 cores."""
    x = np.asarray(x, np.float32)
    freqs_cis = np.asarray(freqs_cis, np.float32)
    wq_a = np.asarray(wq_a, np.float32)
    q_norm_w = np.asarray(q_norm_w, np.float32)
    wq_b = np.asarray(wq_b, np.float32)
    wkv_a = np.asarray(wkv_a, np.float32)
    kv_norm_w = np.asarray(kv_norm_w, np.float32)
    wkv_b = np.asarray(wkv_b, np.float32)
    wo = np.asarray(wo, np.float32)

    f16 = np.float16
    # de-interleave perm for rope pairs: [e0..e31, o0..o31]
    perm = np.concatenate([np.arange(0, ROPE, 2), np.arange(1, ROPE, 2)])

    wqb = (wq_b * q_norm_w[:, None] * SCALE).reshape(QL, H, QKD)
    wqb = np.concatenate(
        [wqb[:, :, :NOPE], wqb[:, :, NOPE:][:, :, perm]], axis=2).astype(f16)

    wkva = np.ascontiguousarray(np.concatenate(
        [wkv_a[:, :KVL], wkv_a[:, KVL:][:, perm]], axis=1).astype(f16))

    wkvb = (wkv_b * kv_norm_w[:, None]).reshape(KVL, H, NOPE + VD).astype(f16)
    wkvb_k = wkvb[:, :, :NOPE]
    wkvb_v = wkvb[:, :, NOPE:]

    wqa16 = np.ascontiguousarray(wq_a.astype(f16))

    cos_t = np.ascontiguousarray(freqs_cis[:, :, 0].T)  # [32, S]
    sin_t = np.ascontiguousarray(freqs_cis[:, :, 1].T)

    sig0 = np.arange(S)
    sig1 = np.concatenate([sig0[512:1024], sig0[0:512],
                           sig0[1536:2048], sig0[1024:1536]])
    qpos = {0: np.concatenate([sig0[512:1024], sig0[1536:2048]]),
            1: np.concatenate([sig0[0:512], sig0[1024:1536]])}

    bias_a0 = np.zeros((P, SC_A), np.float32)
    bias_b0 = np.zeros((P, SC_B), np.float32)
    bias_a1 = np.zeros((P, SC_A), np.float32)
    bias_a1[:, 0:4] = NEG
    bias_b1 = np.zeros((P, SC_B), np.float32)
    bias_b1[:, 8:12] = NEG

    in_maps = []
    meta = []
    for c in range(N_CORES):
        b, g, t = c // 4, (c // 2) % 2, c % 2
        sig = sig0 if t == 0 else sig1
        hs = slice(g * HPC, (g + 1) * HPC)
        m = {
            "xT": np.ascontiguousarray(x[b].T[:, sig].astype(f16)),
            "wq_a": wqa16,
            "wq_b": np.ascontiguousarray(
                wqb[:, hs, :].reshape(QL, HPC * QKD)),
            "wkv_a": wkva,
            "wkv_b_k": np.ascontiguousarray(
                wkvb_k[:, hs, :].reshape(KVL, HPC * NOPE)),
            "wkv_b_v": np.ascontiguousarray(
                wkvb_v[:, hs, :].reshape(KVL, HPC * VD)),
            "wo": np.ascontiguousarray(
                wo[g * HPC * VD:(g + 1) * HPC * VD, :].astype(f16)),
            "cosq": np.ascontiguousarray(cos_t[:, qpos[t]]),
            "sinq": np.ascontiguousarray(sin_t[:, qpos[t]]),
            "cosk": np.ascontiguousarray(cos_t[:, sig]),
            "sink": np.ascontiguousarray(sin_t[:, sig]),
            "bias_a": bias_a0 if t == 0 else bias_a1,
            "bias_b": bias_b0 if t == 0 else bias_b1,
        }
        in_maps.append(m)
        meta.append((b, g, t))
    return in_maps, meta


def kernel(**inputs):
    in_maps, meta = _prep_inputs(**inputs)
    if "nc" not in _CACHE:
        _CACHE["nc"] = build_nc()
    nc = _CACHE["nc"]
    res = run_bass_kernel_spmd(nc, in_maps, core_ids=list(range(N_CORES)),
                               **_CACHE.get("run_kwargs", {}))
    _CACHE["last_result"] = res
    out = np.zeros((B, S, D), np.float32)
    for c in range(N_CORES):
        b, g, t = meta[c]
        part = res.results[c]["out"]  # [1024, 2048]
        if t == 0:
            out[b, 512:1024] += part[:512]
            out[b, 1536:2048] += part[512:]
        else:
            out[b, 0:512] += part[:512]
            out[b, 1024:1536] += part[512:]
    return out


# revision 16
# speedup vs baseline: 1.4638x; 1.1082x over previous
"""MLA attention kernel (DeepSeek-style) for 8 Trainium2 NeuronCores.

Sharding: core = b*4 + g*2 + t over (batch b in {0,1}) x (head-group g in
{0,1}: 8 heads each) x (query-fold t in {0,1}).  Each core computes the
full latent pipeline for its batch, q/k/v + attention for its 8 heads and
its 1024 query tokens (two causally-folded 512-blocks), and a partial
output projection; the host sums the two head-group partials.

All tensors flow transposed ([feature-part, token-free]) so no on-chip
transposes are needed: the host supplies x^T per batch, and causal
structure is made SPMD-uniform by permuting the key order per core and
feeding full-block invalidation as per-partition bias columns consumed by
the Exp activation.  Matmul operands are fp16 (1 cyc/row on the PE; all
values are O(1) so fp16's 11-bit mantissa gives ~5e-4 rounding).
"""

from contextlib import ExitStack

import numpy as np

import concourse.bacc as bacc
import concourse.bass as bass
import concourse.tile as tile
from concourse import mybir
from concourse.bass_utils import run_bass_kernel_spmd

# Problem shapes (hardcoded per contest contract)
B, S, D = 2, 2048, 2048
H = 16
QL = 1536  # q lora rank
KVL = 512  # kv lora rank
NOPE = 128
ROPE = 64
VD = 128
QKD = NOPE + ROPE  # 192
EPS = 1e-6
SCALE = QKD ** (-0.5)

HPC = 8         # heads per core
NQ = 1024       # query tokens per core
P = 128
NEG = -30000.0  # additive mask value (exp -> 0)

F32 = mybir.dt.float32
F16 = mybir.dt.float16
EXP = mybir.ActivationFunctionType.Exp

N_CORES = 8
SC_A = 8   # key 128-chunks for query block a
SC_B = 16  # key 128-chunks for query block b

ND = D // P        # 16
NRQ = QL // P      # 12
NRKV = KVL // P    # 4
HW = ROPE // 2     # 32

_CACHE = {}


def _rope(nc, pool, out_ap, ps, cos_ap, sin_ap, n):
    """rows 0:32 = even pair elems, 32:64 = odd.
    out[0:32] = e*cos - o*sin ; out[32:64] = e*sin + o*cos."""
    e = ps[0:HW, :]
    o = ps[HW:ROPE, :]
    t1 = pool.tile([HW, n], F32, tag="rp1", name="t1")
    nc.vector.tensor_mul(t1[:], e, cos_ap)
    t2 = pool.tile([HW, n], F32, tag="rp2", name="t2")
    nc.vector.tensor_mul(t2[:], o, sin_ap)
    nc.vector.tensor_sub(out_ap[0:HW, :], t1[:], t2[:])
    t3 = pool.tile([HW, n], F32, tag="rp3", name="t3")
    nc.vector.tensor_mul(t3[:], e, sin_ap)
    t4 = pool.tile([HW, n], F32, tag="rp4", name="t4")
    nc.vector.tensor_mul(t4[:], o, cos_ap)
    nc.vector.tensor_add(out_ap[HW:ROPE, :], t3[:], t4[:])


def build_nc():
    nc = bacc.Bacc("TRN2", target_bir_lowering=False, debug=False,
                   num_devices=N_CORES)

    def inp(name, shape, dt=F32):
        return nc.dram_tensor(name, shape, dt, kind="ExternalInput").ap()

    xT = inp("xT", [D, S], F16)
    wqa = inp("wq_a", [D, QL], F16)
    wqb = inp("wq_b", [QL, HPC * QKD], F16)
    wkva = inp("wkv_a", [D, KVL + ROPE], F16)
    wkvbk = inp("wkv_b_k", [KVL, HPC * NOPE], F16)
    wkvbv = inp("wkv_b_v", [KVL, HPC * VD], F16)
    wo = inp("wo", [HPC * VD, D], F16)
    cosq = inp("cosq", [HW, NQ])
    sinq = inp("sinq", [HW, NQ])
    cosk = inp("cosk", [HW, S])
    sink = inp("sink", [HW, S])
    bias_a = inp("bias_a", [P, SC_A])
    bias_b = inp("bias_b", [P, SC_B])
    out = nc.dram_tensor("out", [NQ, D], F32, kind="ExternalOutput").ap()

    with tile.TileContext(nc) as tc, ExitStack() as ctx, \
            nc.allow_low_precision(reason="fp16 matmul pipeline"):
        const = ctx.enter_context(tc.tile_pool(name="const", bufs=1))
        ones_cf = const.tile([P, 1], F32, tag="ones_cf")
        nc.vector.memset(ones_cf[:], 1.0)
        ones_c = const.tile([P, 1], F16, tag="ones_c")
        nc.vector.tensor_copy(ones_c[:], ones_cf[:])
        ones_rf = const.tile([1, P], F32, tag="ones_rf")
        nc.vector.memset(ones_rf[:], 1.0)
        ones_r = const.tile([1, P], F16, tag="ones_r")
        nc.vector.tensor_copy(ones_r[:], ones_rf[:])
        # multiplicative staircase masks (1 keep / 0 drop), applied post-exp
        stairs = []
        for j in range(4):
            st = const.tile([P, 512], F32, tag=f"stair{j}", name=f"st{j}")
            nc.vector.memset(st[:], 1.0)
            # keep 1 where f - p - 128j >= 0 else 0
            nc.gpsimd.affine_select(
                out=st[:], in_=st[:], compare_op=mybir.AluOpType.is_ge,
                fill=0.0, base=-128 * j, pattern=[[1, 512]],
                channel_multiplier=-1)
            stairs.append(st)
        bias_a_sb = const.tile([P, SC_A], F32, tag="bias_a")
        nc.sync.dma_start(bias_a_sb[:], bias_a[:])
        bias_b_sb = const.tile([P, SC_B], F32, tag="bias_b")
        nc.sync.dma_start(bias_b_sb[:], bias_b[:])
        eps_t = const.tile([1, 1], F32, tag="eps")
        nc.vector.memset(eps_t[:], EPS)

        # persistent: kv latents + k_pe (rows 64:128 duplicate rows 0:64 so
        # the scores matmul base-partition matches odd heads' packed qTp)
        latA = ctx.enter_context(tc.tile_pool(name="latA", bufs=1))
        kvT = [latA.tile([P, S], F16, tag=f"kvT{i}", name=f"kvT{i}")
               for i in range(NRKV)]
        kpe = latA.tile([2 * ROPE, S], F16, tag="kpe")

        # packed cq latent: 24 [128,512] slices (rc, tbq) in 8 tiles;
        # reused as oTn after phase 3
        latQ = ctx.enter_context(tc.tile_pool(name="latQ", bufs=1))
        cqPk = [latQ.tile([P, 1536], F16, tag=f"cqPk{i}", name=f"cqPk{i}")
                for i in range(8)]

        def cq_slice(rc, tbq):
            idx = rc * 2 + tbq
            t, c = idx // 3, (idx % 3) * 512
            return cqPk[t][:, c:c + 512]

        oTn = [cqPk[h][:, 0:NQ] for h in range(HPC)]

        ps_main = ctx.enter_context(
            tc.tile_pool(name="ps_main", bufs=3, space="PSUM"))
        ps_small = ctx.enter_context(
            tc.tile_pool(name="ps_small", bufs=2, space="PSUM"))

        def mm_chain(ps_ap, pairs):
            n = len(pairs)
            for i, (lh, rh) in enumerate(pairs):
                nc.tensor.matmul(ps_ap, lh, rh,
                                 start=(i == 0), stop=(i == n - 1))

        # ---------- Phase 1a: KV A-proj over 4 key strips ----------
        with ExitStack() as p1:
            tabk = p1.enter_context(tc.tile_pool(name="tabk", bufs=2))
            ropep = p1.enter_context(tc.tile_pool(name="ropep", bufs=2))
            wkva_p = p1.enter_context(tc.tile_pool(name="wkva", bufs=1))
            wkva_sb = []
            for dc in range(ND):
                wt = wkva_p.tile([P, KVL + ROPE], F16, tag=f"wkva{dc}",
                                 name=f"wkva{dc}")
                nc.sync.dma_start(wt[:], wkva[dc * P:(dc + 1) * P, :])
                wkva_sb.append(wt)
            xt_p = p1.enter_context(tc.tile_pool(name="xt", bufs=2))
            for hs in range(4):
                sl = slice(hs * 512, (hs + 1) * 512)
                xts = []
                for dc in range(ND):
                    xt = xt_p.tile([P, 512], F16, tag=f"xt{dc}",
                                   name=f"xt{dc}")
                    nc.sync.dma_start(xt[:], xT[dc * P:(dc + 1) * P, sl])
                    xts.append(xt)
                for oc in range(NRKV):
                    ps = ps_main.tile([P, 512], F32, tag="ps", name="ps1")
                    mm_chain(ps[:], [
                        (wkva_sb[dc][:, oc * P:(oc + 1) * P], xts[dc][:])
                        for dc in range(ND)])
                    nc.vector.tensor_copy(kvT[oc][:, sl], ps[:])
                psp = ps_main.tile([ROPE, 512], F32, tag="ps", name="ps1p")
                mm_chain(psp[:], [
                    (wkva_sb[dc][:, KVL:KVL + ROPE], xts[dc][:])
                    for dc in range(ND)])
                ck = tabk.tile([HW, 512], F32, tag="cosk", name="ck")
                nc.sync.dma_start(ck[:], cosk[:, sl])
                sk = tabk.tile([HW, 512], F32, tag="sink", name="sk")
                nc.sync.dma_start(sk[:], sink[:, sl])
                _rope(nc, ropep, kpe[0:ROPE, sl], psp, ck[:], sk[:], 512)
                nc.sync.dma_start(kpe[ROPE:2 * ROPE, sl], kpe[0:ROPE, sl])

        # ---------- Phase 1b: Q A-proj (full wq_a, 2 query strips) --------
        with ExitStack() as p1b:
            wqa_p = p1b.enter_context(tc.tile_pool(name="wqa", bufs=1))
            xtq_p = p1b.enter_context(tc.tile_pool(name="xtq", bufs=2))
            wqa_sb = []
            for dc in range(ND):
                wt = wqa_p.tile([P, QL], F16, tag=f"wqa{dc}",
                                name=f"wqa{dc}")
                nc.sync.dma_start(wt[:], wqa[dc * P:(dc + 1) * P, :])
                wqa_sb.append(wt)
            for hs in range(2):
                lo = 512 if hs == 0 else 1536
                xts = []
                for dc in range(ND):
                    xt = xtq_p.tile([P, 512], F16, tag=f"xtq{dc}",
                                    name=f"xtq{dc}")
                    nc.sync.dma_start(
                        xt[:], xT[dc * P:(dc + 1) * P, lo:lo + 512])
                    xts.append(xt)
                for oc in range(NRQ):
                    ps = ps_main.tile([P, 512], F32, tag="ps", name="ps1b")
                    mm_chain(ps[:], [
                        (wqa_sb[dc][:, oc * P:(oc + 1) * P], xts[dc][:])
                        for dc in range(ND)])
                    nc.vector.tensor_copy(cq_slice(oc, hs), ps[:])

        # ---------- Phase 2: RMS norms (transposed layout) ----------
        with ExitStack() as p2:
            sqp = p2.enter_context(tc.tile_pool(name="sq", bufs=3))
            for which, nrc, ntb, nfeat in (
                    (0, NRKV, 4, KVL), (1, NRQ, 2, QL)):
                def sl_of(oc, tb):
                    if which == 0:
                        return kvT[oc][:, tb * 512:(tb + 1) * 512]
                    return cq_slice(oc, tb)
                for tb in range(ntb):
                    pss = ps_small.tile([1, 512], F32, tag="pss", name="pss")
                    for oc in range(nrc):
                        sq = sqp.tile([P, 512], F16, tag="sq", name="sq")
                        nc.vector.tensor_mul(sq[:], sl_of(oc, tb),
                                             sl_of(oc, tb))
                        nc.tensor.matmul(pss[:], ones_c[:], sq[:],
                                         start=(oc == 0), stop=(oc == nrc - 1))
                    sd = sqp.tile([1, 512], F16, tag="sd", name="sd")
                    nc.scalar.activation(
                        sd[:], pss[:], mybir.ActivationFunctionType.Sqrt,
                        bias=eps_t[:], scale=1.0 / nfeat)
                    psb = ps_main.tile([P, 512], F32, tag="ps", name="psb")
                    nc.tensor.matmul(psb[:], ones_r[:], sd[:],
                                     start=True, stop=True)
                    rb = sqp.tile([P, 512], F32, tag="rb", name="rb")
                    nc.vector.reciprocal_approx_fast(rb[:], psb[:])
                    for oc in range(nrc):
                        nc.vector.tensor_mul(sl_of(oc, tb), sl_of(oc, tb),
                                             rb[:])

        # ---------- Phase 3: qT for all heads ----------
        latQT = ctx.enter_context(tc.tile_pool(name="latQT", bufs=1))
        qTn = [latQT.tile([P, NQ], F16, tag=f"qTn{h}", name=f"qTn{h}")
               for h in range(HPC)]
        qTpk = [latQT.tile([P, NQ], F16, tag=f"qTpk{i}", name=f"qTpk{i}")
                for i in range(HPC // 2)]
        qTp = [qTpk[h // 2][(h % 2) * ROPE:(h % 2) * ROPE + ROPE, :]
               for h in range(HPC)]
        with ExitStack() as p3:
            tabq = p3.enter_context(tc.tile_pool(name="tabq", bufs=1))
            cq_sb = tabq.tile([HW, NQ], F32, tag="cosq")
            nc.sync.dma_start(cq_sb[:], cosq[:])
            sq_sb = tabq.tile([HW, NQ], F32, tag="sinq")
            nc.sync.dma_start(sq_sb[:], sinq[:])
            ropep3 = p3.enter_context(tc.tile_pool(name="ropep3", bufs=2))
            wqb_p = p3.enter_context(tc.tile_pool(name="wqb", bufs=3))
            for h in range(HPC):
                wqb_sb = []
                for rc in range(NRQ):
                    wt = wqb_p.tile([P, QKD], F16, tag=f"wqb{rc}",
                                    name=f"wqb{rc}")
                    nc.sync.dma_start(
                        wt[:], wqb[rc * P:(rc + 1) * P, h * QKD:(h + 1) * QKD])
                    wqb_sb.append(wt)
                for tbq in range(2):
                    sl = slice(tbq * 512, (tbq + 1) * 512)
                    ps = ps_main.tile([P, 512], F32, tag="ps", name="ps3")
                    mm_chain(ps[:], [
                        (wqb_sb[rc][:, :NOPE], cq_slice(rc, tbq))
                        for rc in range(NRQ)])
                    nc.vector.tensor_copy(qTn[h][:, sl], ps[:])
                    psp = ps_main.tile([ROPE, 512], F32, tag="ps", name="ps3p")
                    mm_chain(psp[:], [
                        (wqb_sb[rc][:, NOPE:QKD], cq_slice(rc, tbq))
                        for rc in range(NRQ)])
                    _rope(nc, ropep3, qTp[h][:, sl], psp,
                          cq_sb[:, sl], sq_sb[:, sl], 512)

        # ---------- Phase 4: attention per head-pair ----------
        with ExitStack() as p4:
            kt_p = p4.enter_context(tc.tile_pool(name="kt", bufs=4))
            v_p = p4.enter_context(tc.tile_pool(name="v", bufs=2))
            wk_p = p4.enter_context(tc.tile_pool(name="wkvb", bufs=2))
            work = p4.enter_context(tc.tile_pool(name="work", bufs=4))
            ps_o = p4.enter_context(
                tc.tile_pool(name="ps_o", bufs=3, space="PSUM"))
            for hp in range(HPC // 2):
                heads = (2 * hp, 2 * hp + 1)
                kT = {}
                for h in heads:
                    wk_sb = []
                    for rc in range(NRKV):
                        wt = wk_p.tile([P, NOPE], F16, tag=f"wkvbk{rc}",
                                       name=f"wkk{rc}")
                        nc.sync.dma_start(
                            wt[:], wkvbk[rc * P:(rc + 1) * P,
                                         h * NOPE:(h + 1) * NOPE])
                        wk_sb.append(wt)
                    kt = kt_p.tile([P, S], F16, tag="kt", name=f"kt{h}")
                    for tb in range(4):
                        sl = slice(tb * 512, (tb + 1) * 512)
                        ps = ps_main.tile([P, 512], F32, tag="ps", name="ps4k")
                        mm_chain(ps[:], [(wk_sb[rc][:], kvT[rc][:, sl])
                                         for rc in range(NRKV)])
                        nc.vector.tensor_copy(kt[:, sl], ps[:])
                    kT[h] = kt
                wv_sb = []
                for rc in range(NRKV):
                    wt = wk_p.tile([P, 2 * VD], F16, tag=f"wkvbv{rc}",
                                   name=f"wkv{rc}")
                    nc.sync.dma_start(
                        wt[:], wkvbv[rc * P:(rc + 1) * P,
                                     heads[0] * VD:(heads[0] + 2) * VD])
                    wv_sb.append(wt)
                vt = v_p.tile([P, 16 * 2 * VD], F16, tag="vt", name="vt")
                for tk in range(16):
                    ps = ps_main.tile([P, 2 * VD], F32, tag="ps", name="ps4v")
                    mm_chain(ps[:], [
                        (kvT[rc][:, tk * P:(tk + 1) * P], wv_sb[rc][:])
                        for rc in range(NRKV)])
                    nc.vector.tensor_copy(
                        vt[:, tk * 2 * VD:(tk + 1) * 2 * VD], ps[:])

                for h in heads:
                    hv = h % 2
                    for qb, (nsc, bias_sb) in enumerate(
                            ((SC_A, bias_a_sb), (SC_B, bias_b_sb))):
                        q0 = qb * 512
                        oT = ps_o.tile([P, 512], F32, tag="oT", name="oT")
                        pss = ps_small.tile([1, 512], F32, tag="pss",
                                            name="pss4")
                        for sc in range(nsc):
                            sps = ps_main.tile([P, 512], F32, tag="ps",
                                               name="ps4s")
                            nc.tensor.matmul(
                                sps[:], kT[h][:, sc * P:(sc + 1) * P],
                                qTn[h][:, q0:q0 + 512],
                                start=True, stop=False)
                            nc.tensor.matmul(
                                sps[:],
                                kpe[hv * ROPE:(hv + 1) * ROPE,
                                    sc * P:(sc + 1) * P],
                                qTp[h][:, q0:q0 + 512],
                                start=False, stop=True)
                            pt = work.tile([P, 512], F16, tag="pt", name="pt")
                            jd = sc - (nsc - 4)
                            if jd >= 0:
                                nc.scalar.activation(pt[:], sps[:], EXP)
                                nc.vector.tensor_mul(pt[:], pt[:],
                                                     stairs[jd][:])
                            else:
                                nc.scalar.activation(
                                    pt[:], sps[:], EXP,
                                    bias=bias_sb[:, sc:sc + 1])
                            nc.tensor.matmul(pss[:], ones_c[:], pt[:],
                                             start=(sc == 0),
                                             stop=(sc == nsc - 1))
                            nc.tensor.matmul(
                                oT[:],
                                vt[:, sc * 2 * VD + hv * VD:
                                   sc * 2 * VD + (hv + 1) * VD],
                                pt[:], start=(sc == 0),
                                stop=(sc == nsc - 1))
                        ss = work.tile([1, 512], F16, tag="ss", name="ss")
                        nc.scalar.copy(ss[:], pss[:])
                        psb = ps_main.tile([P, 512], F32, tag="ps",
                                           name="ps4b")
                        nc.tensor.matmul(psb[:], ones_r[:], ss[:],
                                         start=True, stop=True)
                        rb = work.tile([P, 512], F32, tag="rb", name="rb")
                        nc.vector.reciprocal_approx_fast(rb[:], psb[:])
                        nc.vector.tensor_mul(oTn[h][:, q0:q0 + 512],
                                             oT[:], rb[:])

        # ---------- Phase 5: output projection (wo aliases qTn/kvT) -------
        with ExitStack() as p5:
            os_p = p5.enter_context(tc.tile_pool(name="os", bufs=4))
            wo_half = []  # per head: (half0 ap, half1 ap), each [128, 1024]
            for h in range(HPC):
                h0 = qTn[h][:, :]
                h1 = kvT[h // 2][:, (h % 2) * 1024:(h % 2) * 1024 + 1024]
                nc.sync.dma_start(h0, wo[h * P:(h + 1) * P, 0:1024])
                nc.sync.dma_start(h1, wo[h * P:(h + 1) * P, 1024:2048])
                wo_half.append((h0, h1))
            for tk in range(NQ // P):
                for dcb in range(4):
                    ps = ps_main.tile([P, 512], F32, tag="ps", name="ps5")
                    for h in range(HPC):
                        src = wo_half[h][dcb // 2]
                        rh = src[:, (dcb % 2) * 512:(dcb % 2) * 512 + 512]
                        nc.tensor.matmul(
                            ps[:], oTn[h][:, tk * P:(tk + 1) * P], rh,
                            start=(h == 0), stop=(h == HPC - 1))
                    ot = os_p.tile([P, 512], F32, tag="ot", name="ot")
                    nc.scalar.copy(ot[:], ps[:])
                    nc.sync.dma_start(
                        out[tk * P:(tk + 1) * P,
                            dcb * 512:(dcb + 1) * 512], ot[:])

    nc.compile()
    return nc


def _prep_inputs(x, freqs_cis, wq_a, q_norm_w, wq_b, wkv_a, kv_norm_w,
                 wkv_b, wo):
    """Host-side shard prep. Returns (in_maps, meta) for 8 cores."""
    x = np.asarray(x, np.float32)
    freqs_cis = np.asarray(freqs_cis, np.float32)
    wq_a = np.asarray(wq_a, np.float32)
    q_norm_w = np.asarray(q_norm_w, np.float32)
    wq_b = np.asarray(wq_b, np.float32)
    wkv_a = np.asarray(wkv_a, np.float32)
    kv_norm_w = np.asarray(kv_norm_w, np.float32)
    wkv_b = np.asarray(wkv_b, np.float32)
    wo = np.asarray(wo, np.float32)

    f16 = np.float16
    # de-interleave perm for rope pairs: [e0..e31, o0..o31]
    perm = np.concatenate([np.arange(0, ROPE, 2), np.arange(1, ROPE, 2)])

    wqb = (wq_b * q_norm_w[:, None] * SCALE).reshape(QL, H, QKD)
    wqb = np.concatenate(
        [wqb[:, :, :NOPE], wqb[:, :, NOPE:][:, :, perm]], axis=2).astype(f16)

    wkva = np.ascontiguousarray(np.concatenate(
        [wkv_a[:, :KVL], wkv_a[:, KVL:][:, perm]], axis=1).astype(f16))

    wkvb = (wkv_b * kv_norm_w[:, None]).reshape(KVL, H, NOPE + VD).astype(f16)
    wkvb_k = wkvb[:, :, :NOPE]
    wkvb_v = wkvb[:, :, NOPE:]

    wqa16 = np.ascontiguousarray(wq_a.astype(f16))

    cos_t = np.ascontiguousarray(freqs_cis[:, :, 0].T)  # [32, S]
    sin_t = np.ascontiguousarray(freqs_cis[:, :, 1].T)

    sig0 = np.arange(S)
    sig1 = np.concatenate([sig0[512:1024], sig0[0:512],
                           sig0[1536:2048], sig0[1024:1536]])
    qpos = {0: np.concatenate([sig0[512:1024], sig0[1536:2048]]),
            1: np.concatenate([sig0[0:512], sig0[1024:1536]])}

    bias_a0 = np.zeros((P, SC_A), np.float32)
    bias_b0 = np.zeros((P, SC_B), np.float32)
    bias_a1 = np.zeros((P, SC_A), np.float32)
    bias_a1[:, 0:4] = NEG
    bias_b1 = np.zeros((P, SC_B), np.float32)
    bias_b1[:, 8:12] = NEG

    in_maps = []
    meta = []
    for c in range(N_CORES):
        b, g, t = c // 4, (c // 2) % 2, c % 2
        sig = sig0 if t == 0 else sig1
        hs = slice(g * HPC, (g + 1) * HPC)
        m = {
            "xT": np.ascontiguousarray(x[b].T[:, sig].astype(f16)),
            "wq_a": wqa16,
            "wq_b": np.ascontiguousarray(
                wqb[:, hs, :].reshape(QL, HPC * QKD)),
            "wkv_a": wkva,
            "wkv_b_k": np.ascontiguousarray(
                wkvb_k[:, hs, :].reshape(KVL, HPC * NOPE)),
            "wkv_b_v": np.ascontiguousarray(
                wkvb_v[:, hs, :].reshape(KVL, HPC * VD)),
            "wo": np.ascontiguousarray(
                wo[g * HPC * VD:(g + 1) * HPC * VD, :].astype(f16)),
            "cosq": np.ascontiguousarray(cos_t[:, qpos[t]]),
            "sinq": np.ascontiguousarray(sin_t[:, qpos[t]]),
            "cosk": np.ascontiguousarray(cos_t[:, sig]),
            "sink": np.ascontiguousarray(sin_t[:, sig]),
            "bias_a": bias_a0 if t == 0 else bias_a1,
            "bias_b": bias_b0 if t == 0 else bias_b1,
        }
        in_maps.append(m)
        meta.append((b, g, t))
    return in_maps, meta


def kernel(**inputs):
    in_maps, meta = _prep_inputs(**inputs)
    if "nc" not in _CACHE:
        _CACHE["nc"] = build_nc()
    nc = _CACHE["nc"]
    res = run_bass_kernel_spmd(nc, in_maps, core_ids=list(range(N_CORES)),
                               **_CACHE.get("run_kwargs", {}))
    _CACHE["last_result"] = res
    out = np.zeros((B, S, D), np.float32)
    for c in range(N_CORES):
        b, g, t = meta[c]
        part = res.results[c]["out"]  # [1024, 2048]
        if t == 0:
            out[b, 512:1024] += part[:512]
            out[b, 1536:2048] += part[512:]
        else:
            out[b, 0:512] += part[:512]
            out[b, 1024:1536] += part[512:]
    return out


# revision 17
# speedup vs baseline: 1.4966x; 1.0224x over previous
"""MLA attention kernel (DeepSeek-style) for 8 Trainium2 NeuronCores.

Sharding: core = b*4 + g*2 + t over (batch b in {0,1}) x (head-group g in
{0,1}: 8 heads each) x (query-fold t in {0,1}).  Each core computes the
full latent pipeline for its batch, q/k/v + attention for its 8 heads and
its 1024 query tokens (two causally-folded 512-blocks), and a partial
output projection; the host sums the two head-group partials.

All tensors flow transposed ([feature-part, token-free]) so no on-chip
transposes are needed: the host supplies x^T per batch, and causal
structure is made SPMD-uniform by permuting the key order per core and
feeding full-block invalidation as per-partition bias columns consumed by
the Exp activation.  Matmul operands are fp16 (1 cyc/row on the PE; all
values are O(1) so fp16's 11-bit mantissa gives ~5e-4 rounding).
"""

from contextlib import ExitStack

import numpy as np

import concourse.bacc as bacc
import concourse.bass as bass
import concourse.tile as tile
from concourse import mybir
from concourse.bass_utils import run_bass_kernel_spmd

# Problem shapes (hardcoded per contest contract)
B, S, D = 2, 2048, 2048
H = 16
QL = 1536  # q lora rank
KVL = 512  # kv lora rank
NOPE = 128
ROPE = 64
VD = 128
QKD = NOPE + ROPE  # 192
EPS = 1e-6
SCALE = QKD ** (-0.5)

HPC = 8         # heads per core
NQ = 1024       # query tokens per core
P = 128
NEG = -30000.0  # additive mask value (exp -> 0)

F32 = mybir.dt.float32
F16 = mybir.dt.float16
EXP = mybir.ActivationFunctionType.Exp

N_CORES = 8
SC_A = 8   # key 128-chunks for query block a
SC_B = 16  # key 128-chunks for query block b

ND = D // P        # 16
NRQ = QL // P      # 12
NRKV = KVL // P    # 4
HW = ROPE // 2     # 32

_CACHE = {}


def _rope(nc, pool, out_ap, ps, cos_ap, sin_ap, n):
    """rows 0:32 = even pair elems, 32:64 = odd.
    out[0:32] = e*cos - o*sin ; out[32:64] = e*sin + o*cos."""
    e = ps[0:HW, :]
    o = ps[HW:ROPE, :]
    t1 = pool.tile([HW, n], F32, tag="rp1", name="t1")
    nc.vector.tensor_mul(t1[:], e, cos_ap)
    t2 = pool.tile([HW, n], F32, tag="rp2", name="t2")
    nc.vector.tensor_mul(t2[:], o, sin_ap)
    nc.vector.tensor_sub(out_ap[0:HW, :], t1[:], t2[:])
    t3 = pool.tile([HW, n], F32, tag="rp3", name="t3")
    nc.vector.tensor_mul(t3[:], e, sin_ap)
    t4 = pool.tile([HW, n], F32, tag="rp4", name="t4")
    nc.vector.tensor_mul(t4[:], o, cos_ap)
    nc.vector.tensor_add(out_ap[HW:ROPE, :], t3[:], t4[:])


def build_nc():
    nc = bacc.Bacc("TRN2", target_bir_lowering=False, debug=False,
                   num_devices=N_CORES)

    def inp(name, shape, dt=F32):
        return nc.dram_tensor(name, shape, dt, kind="ExternalInput").ap()

    xT = inp("xT", [D, S], F16)
    wqa = inp("wq_a", [D, QL], F16)
    wqb = inp("wq_b", [QL, HPC * QKD], F16)
    wkva = inp("wkv_a", [D, KVL + ROPE], F16)
    wkvbk = inp("wkv_b_k", [KVL, HPC * NOPE], F16)
    wkvbv = inp("wkv_b_v", [KVL, HPC * VD], F16)
    wo = inp("wo", [HPC * VD, D], F16)
    cosq = inp("cosq", [HW, NQ])
    sinq = inp("sinq", [HW, NQ])
    cosk = inp("cosk", [HW, S])
    sink = inp("sink", [HW, S])
    bias_a = inp("bias_a", [P, SC_A])
    bias_b = inp("bias_b", [P, SC_B])
    out = nc.dram_tensor("out", [NQ, D], F32, kind="ExternalOutput").ap()

    with tile.TileContext(nc) as tc, ExitStack() as ctx, \
            nc.allow_low_precision(reason="fp16 matmul pipeline"):
        const = ctx.enter_context(tc.tile_pool(name="const", bufs=1))
        ones_cf = const.tile([P, 1], F32, tag="ones_cf")
        nc.vector.memset(ones_cf[:], 1.0)
        ones_c = const.tile([P, 1], F16, tag="ones_c")
        nc.vector.tensor_copy(ones_c[:], ones_cf[:])
        ones_rf = const.tile([1, P], F32, tag="ones_rf")
        nc.vector.memset(ones_rf[:], 1.0)
        ones_r = const.tile([1, P], F16, tag="ones_r")
        nc.vector.tensor_copy(ones_r[:], ones_rf[:])
        # multiplicative staircase masks (1 keep / 0 drop), applied post-exp
        stairs = []
        for j in range(4):
            st = const.tile([P, 512], F32, tag=f"stair{j}", name=f"st{j}")
            nc.vector.memset(st[:], 1.0)
            # keep 1 where f - p - 128j >= 0 else 0
            nc.gpsimd.affine_select(
                out=st[:], in_=st[:], compare_op=mybir.AluOpType.is_ge,
                fill=0.0, base=-128 * j, pattern=[[1, 512]],
                channel_multiplier=-1)
            stairs.append(st)
        bias_a_sb = const.tile([P, SC_A], F32, tag="bias_a")
        nc.sync.dma_start(bias_a_sb[:], bias_a[:])
        bias_b_sb = const.tile([P, SC_B], F32, tag="bias_b")
        nc.sync.dma_start(bias_b_sb[:], bias_b[:])
        eps_t = const.tile([1, 1], F32, tag="eps")
        nc.vector.memset(eps_t[:], EPS)

        # persistent: kv latents + k_pe (rows 64:128 duplicate rows 0:64 so
        # the scores matmul base-partition matches odd heads' packed qTp)
        latA = ctx.enter_context(tc.tile_pool(name="latA", bufs=1))
        kvT = [latA.tile([P, S], F16, tag=f"kvT{i}", name=f"kvT{i}")
               for i in range(NRKV)]
        kpe = latA.tile([2 * ROPE, S], F16, tag="kpe")

        # packed cq latent: 24 [128,512] slices (rc, tbq) in 8 tiles;
        # reused as oTn after phase 3
        latQ = ctx.enter_context(tc.tile_pool(name="latQ", bufs=1))
        cqPk = [latQ.tile([P, 1536], F16, tag=f"cqPk{i}", name=f"cqPk{i}")
                for i in range(8)]

        def cq_slice(rc, tbq):
            idx = rc * 2 + tbq
            t, c = idx // 3, (idx % 3) * 512
            return cqPk[t][:, c:c + 512]

        oTn = [cqPk[h][:, 0:NQ] for h in range(HPC)]

        ps_main = ctx.enter_context(
            tc.tile_pool(name="ps_main", bufs=3, space="PSUM"))
        ps_small = ctx.enter_context(
            tc.tile_pool(name="ps_small", bufs=2, space="PSUM"))

        def mm_chain(ps_ap, pairs):
            n = len(pairs)
            for i, (lh, rh) in enumerate(pairs):
                nc.tensor.matmul(ps_ap, lh, rh,
                                 start=(i == 0), stop=(i == n - 1))

        # ---------- Phase 1: A-projections (KV strips, then Q strips) ----
        with ExitStack() as p1:
            tabk = p1.enter_context(tc.tile_pool(name="tabk", bufs=2))
            ropep = p1.enter_context(tc.tile_pool(name="ropep", bufs=2))
            wkva_p = p1.enter_context(tc.tile_pool(name="wkva", bufs=1))
            wqa_p = p1.enter_context(tc.tile_pool(name="wqa", bufs=1))
            wkva_sb = []
            for dc in range(ND):
                wt = wkva_p.tile([P, KVL + ROPE], F16, tag=f"wkva{dc}",
                                 name=f"wkva{dc}")
                nc.sync.dma_start(wt[:], wkva[dc * P:(dc + 1) * P, :])
                wkva_sb.append(wt)
            wqa_sb = []
            for dc in range(ND):
                wt = wqa_p.tile([P, QL], F16, tag=f"wqa{dc}",
                                name=f"wqa{dc}")
                nc.sync.dma_start(wt[:], wqa[dc * P:(dc + 1) * P, :])
                wqa_sb.append(wt)
            xt_p = p1.enter_context(tc.tile_pool(name="xt", bufs=2))
            for hs in range(4):
                sl = slice(hs * 512, (hs + 1) * 512)
                xts = []
                for dc in range(ND):
                    xt = xt_p.tile([P, 512], F16, tag=f"xt{dc}",
                                   name=f"xt{dc}")
                    nc.sync.dma_start(xt[:], xT[dc * P:(dc + 1) * P, sl])
                    xts.append(xt)
                for oc in range(NRKV):
                    ps = ps_main.tile([P, 512], F32, tag="ps", name="ps1")
                    mm_chain(ps[:], [
                        (wkva_sb[dc][:, oc * P:(oc + 1) * P], xts[dc][:])
                        for dc in range(ND)])
                    nc.vector.tensor_copy(kvT[oc][:, sl], ps[:])
                psp = ps_main.tile([ROPE, 512], F32, tag="ps", name="ps1p")
                mm_chain(psp[:], [
                    (wkva_sb[dc][:, KVL:KVL + ROPE], xts[dc][:])
                    for dc in range(ND)])
                ck = tabk.tile([HW, 512], F32, tag="cosk", name="ck")
                nc.sync.dma_start(ck[:], cosk[:, sl])
                sk = tabk.tile([HW, 512], F32, tag="sink", name="sk")
                nc.sync.dma_start(sk[:], sink[:, sl])
                _rope(nc, ropep, kpe[0:ROPE, sl], psp, ck[:], sk[:], 512)
                nc.sync.dma_start(kpe[ROPE:2 * ROPE, sl], kpe[0:ROPE, sl])

            # Q A-proj: 2 query strips reusing the same xt slots
            for hs in range(2):
                lo = 512 if hs == 0 else 1536
                xts = []
                for dc in range(ND):
                    xt = xt_p.tile([P, 512], F16, tag=f"xt{dc}",
                                   name=f"xtq{dc}")
                    nc.sync.dma_start(
                        xt[:], xT[dc * P:(dc + 1) * P, lo:lo + 512])
                    xts.append(xt)
                for oc in range(NRQ):
                    ps = ps_main.tile([P, 512], F32, tag="ps", name="ps1b")
                    mm_chain(ps[:], [
                        (wqa_sb[dc][:, oc * P:(oc + 1) * P], xts[dc][:])
                        for dc in range(ND)])
                    nc.vector.tensor_copy(cq_slice(oc, hs), ps[:])

        # ---------- Phase 2: RMS norms (transposed layout) ----------
        with ExitStack() as p2:
            sqp = p2.enter_context(tc.tile_pool(name="sq", bufs=3))
            for which, nrc, ntb, nfeat in (
                    (1, NRQ, 2, QL), (0, NRKV, 4, KVL)):
                def sl_of(oc, tb):
                    if which == 0:
                        return kvT[oc][:, tb * 512:(tb + 1) * 512]
                    return cq_slice(oc, tb)
                for tb in range(ntb):
                    pss = ps_small.tile([1, 512], F32, tag="pss", name="pss")
                    for oc in range(nrc):
                        sq = sqp.tile([P, 512], F16, tag="sq", name="sq")
                        nc.vector.tensor_mul(sq[:], sl_of(oc, tb),
                                             sl_of(oc, tb))
                        nc.tensor.matmul(pss[:], ones_c[:], sq[:],
                                         start=(oc == 0), stop=(oc == nrc - 1))
                    sd = sqp.tile([1, 512], F16, tag="sd", name="sd")
                    nc.scalar.activation(
                        sd[:], pss[:], mybir.ActivationFunctionType.Sqrt,
                        bias=eps_t[:], scale=1.0 / nfeat)
                    psb = ps_main.tile([P, 512], F32, tag="ps", name="psb")
                    nc.tensor.matmul(psb[:], ones_r[:], sd[:],
                                     start=True, stop=True)
                    rb = sqp.tile([P, 512], F32, tag="rb", name="rb")
                    nc.vector.reciprocal_approx_fast(rb[:], psb[:])
                    for oc in range(nrc):
                        nc.vector.tensor_mul(sl_of(oc, tb), sl_of(oc, tb),
                                             rb[:])

        # ---------- Phase 3: qT for all heads ----------
        latQT = ctx.enter_context(tc.tile_pool(name="latQT", bufs=1))
        qTn = [latQT.tile([P, NQ], F16, tag=f"qTn{h}", name=f"qTn{h}")
               for h in range(HPC)]
        qTpk = [latQT.tile([P, NQ], F16, tag=f"qTpk{i}", name=f"qTpk{i}")
                for i in range(HPC // 2)]
        qTp = [qTpk[h // 2][(h % 2) * ROPE:(h % 2) * ROPE + ROPE, :]
               for h in range(HPC)]
        with ExitStack() as p3:
            tabq = p3.enter_context(tc.tile_pool(name="tabq", bufs=1))
            cq_sb = tabq.tile([HW, NQ], F32, tag="cosq")
            nc.sync.dma_start(cq_sb[:], cosq[:])
            sq_sb = tabq.tile([HW, NQ], F32, tag="sinq")
            nc.sync.dma_start(sq_sb[:], sinq[:])
            ropep3 = p3.enter_context(tc.tile_pool(name="ropep3", bufs=2))
            wqb_p = p3.enter_context(tc.tile_pool(name="wqb", bufs=3))
            for h in range(HPC):
                wqb_sb = []
                for rc in range(NRQ):
                    wt = wqb_p.tile([P, QKD], F16, tag=f"wqb{rc}",
                                    name=f"wqb{rc}")
                    nc.sync.dma_start(
                        wt[:], wqb[rc * P:(rc + 1) * P, h * QKD:(h + 1) * QKD])
                    wqb_sb.append(wt)
                for tbq in range(2):
                    sl = slice(tbq * 512, (tbq + 1) * 512)
                    ps = ps_main.tile([P, 512], F32, tag="ps", name="ps3")
                    mm_chain(ps[:], [
                        (wqb_sb[rc][:, :NOPE], cq_slice(rc, tbq))
                        for rc in range(NRQ)])
                    nc.vector.tensor_copy(qTn[h][:, sl], ps[:])
                    psp = ps_main.tile([ROPE, 512], F32, tag="ps", name="ps3p")
                    mm_chain(psp[:], [
                        (wqb_sb[rc][:, NOPE:QKD], cq_slice(rc, tbq))
                        for rc in range(NRQ)])
                    _rope(nc, ropep3, qTp[h][:, sl], psp,
                          cq_sb[:, sl], sq_sb[:, sl], 512)

        # ---------- Phase 4: attention per head-pair ----------
        wo_p = ctx.enter_context(tc.tile_pool(name="wo", bufs=1))
        wo_sb = []
        for h in range(HPC):
            wt = wo_p.tile([P, D], F16, tag=f"wo{h}", name=f"wo{h}")
            nc.sync.dma_start(wt[:], wo[h * P:(h + 1) * P, :])
            wo_sb.append(wt)
        with ExitStack() as p4:
            kt_p = p4.enter_context(tc.tile_pool(name="kt", bufs=4))
            v_p = p4.enter_context(tc.tile_pool(name="v", bufs=2))
            wk_p = p4.enter_context(tc.tile_pool(name="wkvb", bufs=2))
            work = p4.enter_context(tc.tile_pool(name="work", bufs=4))
            ps_o = p4.enter_context(
                tc.tile_pool(name="ps_o", bufs=3, space="PSUM"))
            for hp in range(HPC // 2):
                heads = (2 * hp, 2 * hp + 1)
                kT = {}
                for h in heads:
                    wk_sb = []
                    for rc in range(NRKV):
                        wt = wk_p.tile([P, NOPE], F16, tag=f"wkvbk{rc}",
                                       name=f"wkk{rc}")
                        nc.sync.dma_start(
                            wt[:], wkvbk[rc * P:(rc + 1) * P,
                                         h * NOPE:(h + 1) * NOPE])
                        wk_sb.append(wt)
                    kt = kt_p.tile([P, S], F16, tag="kt", name=f"kt{h}")
                    for tb in range(4):
                        sl = slice(tb * 512, (tb + 1) * 512)
                        ps = ps_main.tile([P, 512], F32, tag="ps", name="ps4k")
                        mm_chain(ps[:], [(wk_sb[rc][:], kvT[rc][:, sl])
                                         for rc in range(NRKV)])
                        nc.vector.tensor_copy(kt[:, sl], ps[:])
                    kT[h] = kt
                wv_sb = []
                for rc in range(NRKV):
                    wt = wk_p.tile([P, 2 * VD], F16, tag=f"wkvbv{rc}",
                                   name=f"wkv{rc}")
                    nc.sync.dma_start(
                        wt[:], wkvbv[rc * P:(rc + 1) * P,
                                     heads[0] * VD:(heads[0] + 2) * VD])
                    wv_sb.append(wt)
                vt = v_p.tile([P, 16 * 2 * VD], F16, tag="vt", name="vt")
                for tk in range(16):
                    ps = ps_main.tile([P, 2 * VD], F32, tag="ps", name="ps4v")
                    mm_chain(ps[:], [
                        (kvT[rc][:, tk * P:(tk + 1) * P], wv_sb[rc][:])
                        for rc in range(NRKV)])
                    nc.vector.tensor_copy(
                        vt[:, tk * 2 * VD:(tk + 1) * 2 * VD], ps[:])

                for h in heads:
                    hv = h % 2
                    for qb, (nsc, bias_sb) in enumerate(
                            ((SC_A, bias_a_sb), (SC_B, bias_b_sb))):
                        q0 = qb * 512
                        oT = ps_o.tile([P, 512], F32, tag="oT", name="oT")
                        pss = ps_small.tile([1, 512], F32, tag="pss",
                                            name="pss4")
                        for sc in range(nsc):
                            sps = ps_main.tile([P, 512], F32, tag="ps",
                                               name="ps4s")
                            nc.tensor.matmul(
                                sps[:], kT[h][:, sc * P:(sc + 1) * P],
                                qTn[h][:, q0:q0 + 512],
                                start=True, stop=False)
                            nc.tensor.matmul(
                                sps[:],
                                kpe[hv * ROPE:(hv + 1) * ROPE,
                                    sc * P:(sc + 1) * P],
                                qTp[h][:, q0:q0 + 512],
                                start=False, stop=True)
                            pt = work.tile([P, 512], F16, tag="pt", name="pt")
                            jd = sc - (nsc - 4)
                            if jd >= 0:
                                nc.scalar.activation(pt[:], sps[:], EXP)
                                nc.vector.tensor_mul(pt[:], pt[:],
                                                     stairs[jd][:])
                            else:
                                nc.scalar.activation(
                                    pt[:], sps[:], EXP,
                                    bias=bias_sb[:, sc:sc + 1])
                            nc.tensor.matmul(pss[:], ones_c[:], pt[:],
                                             start=(sc == 0),
                                             stop=(sc == nsc - 1))
                            nc.tensor.matmul(
                                oT[:],
                                vt[:, sc * 2 * VD + hv * VD:
                                   sc * 2 * VD + (hv + 1) * VD],
                                pt[:], start=(sc == 0),
                                stop=(sc == nsc - 1))
                        ss = work.tile([1, 512], F16, tag="ss", name="ss")
                        nc.scalar.copy(ss[:], pss[:])
                        psb = ps_main.tile([P, 512], F32, tag="ps",
                                           name="ps4b")
                        nc.tensor.matmul(psb[:], ones_r[:], ss[:],
                                         start=True, stop=True)
                        rb = work.tile([P, 512], F32, tag="rb", name="rb")
                        nc.vector.reciprocal_approx_fast(rb[:], psb[:])
                        nc.vector.tensor_mul(oTn[h][:, q0:q0 + 512],
                                             oT[:], rb[:])

        # ---------- Phase 5: output projection (wo aliases qTn/kvT) -------
        with ExitStack() as p5:
            os_p = p5.enter_context(tc.tile_pool(name="os", bufs=4))
            for tk in range(NQ // P):
                for dcb in range(4):
                    ps = ps_main.tile([P, 512], F32, tag="ps", name="ps5")
                    for h in range(HPC):
                        rh = wo_sb[h][:, dcb * 512:(dcb + 1) * 512]
                        nc.tensor.matmul(
                            ps[:], oTn[h][:, tk * P:(tk + 1) * P], rh,
                            start=(h == 0), stop=(h == HPC - 1))
                    ot = os_p.tile([P, 512], F32, tag="ot", name="ot")
                    nc.scalar.copy(ot[:], ps[:])
                    nc.sync.dma_start(
                        out[tk * P:(tk + 1) * P,
                            dcb * 512:(dcb + 1) * 512], ot[:])

    nc.compile()
    return nc


def _prep_inputs(x, freqs_cis, wq_a, q_norm_w, wq_b, wkv_a, kv_norm_w,
                 wkv_b, wo):
    """Host-side shard prep. Returns (in_maps, meta) for 8 cores."""
    x = np.asarray(x, np.float32)
    freqs_cis = np.asarray(freqs_cis, np.float32)
    wq_a = np.asarray(wq_a, np.float32)
    q_norm_w = np.asarray(q_norm_w, np.float32)
    wq_b = np.asarray(wq_b, np.float32)
    wkv_a = np.asarray(wkv_a, np.float32)
    kv_norm_w = np.asarray(kv_norm_w, np.float32)
    wkv_b = np.asarray(wkv_b, np.float32)
    wo = np.asarray(wo, np.float32)

    f16 = np.float16
    # de-interleave perm for rope pairs: [e0..e31, o0..o31]
    perm = np.concatenate([np.arange(0, ROPE, 2), np.arange(1, ROPE, 2)])

    wqb = (wq_b * q_norm_w[:, None] * SCALE).reshape(QL, H, QKD)
    wqb = np.concatenate(
        [wqb[:, :, :NOPE], wqb[:, :, NOPE:][:, :, perm]], axis=2).astype(f16)

    wkva = np.ascontiguousarray(np.concatenate(
        [wkv_a[:, :KVL], wkv_a[:, KVL:][:, perm]], axis=1).astype(f16))

    wkvb = (wkv_b * kv_norm_w[:, None]).reshape(KVL, H, NOPE + VD).astype(f16)
    wkvb_k = wkvb[:, :, :NOPE]
    wkvb_v = wkvb[:, :, NOPE:]

    wqa16 = np.ascontiguousarray(wq_a.astype(f16))

    cos_t = np.ascontiguousarray(freqs_cis[:, :, 0].T)  # [32, S]
    sin_t = np.ascontiguousarray(freqs_cis[:, :, 1].T)

    sig0 = np.arange(S)
    sig1 = np.concatenate([sig0[512:1024], sig0[0:512],
                           sig0[1536:2048], sig0[1024:1536]])
    qpos = {0: np.concatenate([sig0[512:1024], sig0[1536:2048]]),
            1: np.concatenate([sig0[0:512], sig0[1024:1536]])}

    bias_a0 = np.zeros((P, SC_A), np.float32)
    bias_b0 = np.zeros((P, SC_B), np.float32)
    bias_a1 = np.zeros((P, SC_A), np.float32)
    bias_a1[:, 0:4] = NEG
    bias_b1 = np.zeros((P, SC_B), np.float32)
    bias_b1[:, 8:12] = NEG

    in_maps = []
    meta = []
    for c in range(N_CORES):
        b, g, t = c // 4, (c // 2) % 2, c % 2
        sig = sig0 if t == 0 else sig1
        hs = slice(g * HPC, (g + 1) * HPC)
        m = {
            "xT": np.ascontiguousarray(x[b].T[:, sig].astype(f16)),
            "wq_a": wqa16,
            "wq_b": np.ascontiguousarray(
                wqb[:, hs, :].reshape(QL, HPC * QKD)),
            "wkv_a": wkva,
            "wkv_b_k": np.ascontiguousarray(
                wkvb_k[:, hs, :].reshape(KVL, HPC * NOPE)),
            "wkv_b_v": np.ascontiguousarray(
                wkvb_v[:, hs, :].reshape(KVL, HPC * VD)),
            "wo": np.ascontiguousarray(
                wo[g * HPC * VD:(g + 1) * HPC * VD, :].astype(f16)),
            "cosq": np.ascontiguousarray(cos_t[:, qpos[t]]),
            "sinq": np.ascontiguousarray(sin_t[:, qpos[t]]),
            "cosk": np.ascontiguousarray(cos_t[:, sig]),
            "sink": np.ascontiguousarray(sin_t[:, sig]),
            "bias_a": bias_a0 if t == 0 else bias_a1,
            "bias_b": bias_b0 if t == 0 else bias_b1,
        }
        in_maps.append(m)
        meta.append((b, g, t))
    return in_maps, meta


def kernel(**inputs):
    in_maps, meta = _prep_inputs(**inputs)
    if "nc" not in _CACHE:
        _CACHE["nc"] = build_nc()
    nc = _CACHE["nc"]
    res = run_bass_kernel_spmd(nc, in_maps, core_ids=list(range(N_CORES)),
                               **_CACHE.get("run_kwargs", {}))
    _CACHE["last_result"] = res
    out = np.zeros((B, S, D), np.float32)
    for c in range(N_CORES):
        b, g, t = meta[c]
        part = res.results[c]["out"]  # [1024, 2048]
        if t == 0:
            out[b, 512:1024] += part[:512]
            out[b, 1536:2048] += part[512:]
        else:
            out[b, 0:512] += part[:512]
            out[b, 1024:1536] += part[512:]
    return out
